# revision 9
# baseline (speedup 1.0000x reference)
"""Trainium2 Bass kernel for nn_EstimatePSF: FFT-based PSF estimation via CG.

Strategy:
- All 2D FFTs/IFFTs expressed as DFT matmuls on the TensorEngine (fp32).
  Rolls/pads/crops are absorbed into precomputed DFT-matrix constants.
- Data-parallel over the 12 (b,c) slices; SPMD over 8 cores, 2 slices per
  core (4 slices duplicated to fill 16 = 8*2 program slots). No collectives.
- All 512x512 spectra live TRANSPOSED ("spectrum layout"); the 31x31 CG
  state stays natural. crop-IFFT swaps lhsT/rhs in its last stage so the
  natural orientation comes back for free.
- r0 computed via linearity: D = bf - lft*xf0 (xf0 = analytic spectrum of
  the uniform init kernel, masked -> real), r0 = cropIFFT(D) - x0.
- The psf2otf imag-mask is computed with max|Im|, max|Re| reductions and
  applied by scaling the imag-term DFT constants by keep (0/1) - exact.

Self-contained: hardcodes shapes (4,3,512,512) f32, psf_size=31.
"""
import sys
import math as _math
import numpy as np

sys.path.insert(0, '/opt/trn_rl_repo')

P = 31
N = 512
EPS32 = 1.1920928955078125e-07
NOPS_T = np.float32(P * P * (2.0 * _math.log2(P)) * EPS32)
T2 = float(np.float32(np.float32(NOPS_T) * np.float32(NOPS_T)))
N_ITER = 10
NCORES = 8
SLICES_PER_CORE = 2


def _to_sb(a):
    """[512, X] row-major -> SBUF layout [128, 4X] (4 row-chunks side by side)."""
    X = a.shape[1]
    return np.ascontiguousarray(
        a.reshape(4, 128, X).transpose(1, 0, 2).reshape(128, 4 * X))


def _make_consts():
    k = np.arange(N)
    ang = -2.0 * np.pi * np.outer(k, k) / N
    Wr = np.cos(ang).astype(np.float32)   # symmetric
    Wi = np.sin(ang).astype(np.float32)
    i31 = np.arange(P) - (P // 2)
    angc = -2.0 * np.pi * np.outer(k, i31) / N   # [512, 31] : Wc
    WcTr = np.cos(angc).astype(np.float32).T.copy()  # [31, 512]
    WcTi = np.sin(angc).astype(np.float32).T.copy()
    angp = 2.0 * np.pi * np.outer(i31, k) / N    # [31, 512]
    Er = np.cos(angp).astype(np.float64)
    Ei = np.sin(angp).astype(np.float64)
    PlTr = (Er / (N * N)).astype(np.float32).T.copy()  # [512, 31]
    PlTi = (Ei / (N * N)).astype(np.float32).T.copy()
    PrTr = Er.astype(np.float32).T.copy()
    PrTi = Ei.astype(np.float32).T.copy()
    with np.errstate(invalid='ignore', divide='ignore'):
        D31 = np.sin(31 * np.pi * k / N) / np.sin(np.pi * k / N)
    D31[0] = 31.0
    xf0 = (np.outer(D31, D31) / (P * P)).astype(np.float32)
    return {
        "wr": _to_sb(Wr), "wi": _to_sb(Wi), "nwi": _to_sb(-Wi),
        "wctr": WcTr, "wcti": WcTi, "nwcti": (-WcTi).copy(),
        "pltr": _to_sb(PlTr), "plti": _to_sb(PlTi), "nplti": _to_sb(-PlTi),
        "prtr": _to_sb(PrTr), "nprti": _to_sb(-PrTi),
        "xf0": _to_sb(xf0),
        "ident": np.eye(128, dtype=np.float32),
    }


_PROGRAM_CACHE = {}


def _build_program(n_iter=N_ITER, stage=99, sub=99):
    from contextlib import ExitStack
    import concourse.bacc as bacc
    import concourse.tile as tile
    from concourse import mybir
    from concourse.alu_op_type import AluOpType

    F32 = mybir.dt.float32
    AX = mybir.AxisListType
    MUL = AluOpType.mult
    ADD = AluOpType.add
    MAX = AluOpType.max

    nc = bacc.Bacc(None, target_bir_lowering=False, debug=False)

    # ---- DRAM ----
    d_in = {}
    for nm in ("bx", "by", "lx", "ly"):
        d_in[nm] = nc.dram_tensor(nm, [SLICES_PER_CORE, 128, 4 * N], F32,
                                  kind="ExternalInput").ap()
    d_c = {}
    for nm, shp in (("wr", [128, 4 * N]), ("wi", [128, 4 * N]),
                    ("nwi", [128, 4 * N]),
                    ("wctr", [P, N]), ("wcti", [P, N]), ("nwcti", [P, N]),
                    ("pltr", [128, 4 * P]), ("plti", [128, 4 * P]),
                    ("nplti", [128, 4 * P]),
                    ("prtr", [128, 4 * P]), ("nprti", [128, 4 * P]),
                    ("xf0", [128, 4 * N]), ("ident", [128, 128])):
        d_c[nm] = nc.dram_tensor(nm, shp, F32, kind="ExternalInput").ap()
    d_out = nc.dram_tensor("out", [SLICES_PER_CORE, P, P], F32,
                           kind="ExternalOutput").ap()

    with tile.TileContext(nc) as tc, ExitStack() as ctx:
        cp = ctx.enter_context(tc.tile_pool(name="consts", bufs=1))
        wp = ctx.enter_context(tc.tile_pool(name="work", bufs=1))
        pmm = ctx.enter_context(tc.tile_pool(name="pmm", bufs=4, space="PSUM"))
        ptc = ctx.enter_context(tc.tile_pool(name="ptc", bufs=2, space="PSUM"))
        psml = ctx.enter_context(tc.tile_pool(name="psml", bufs=2, space="PSUM"))

        # ---- constants to SBUF ----
        c = {}
        for nm in d_c:
            if nm == "xf0":
                continue  # streamed chunk-wise from DRAM
            c[nm] = cp.tile(list(d_c[nm].shape), F32, name=f"c_{nm}")
            nc.sync.dma_start(c[nm][:], d_c[nm][:])
        ones31 = cp.tile([P, P], F32, name="ones31")
        nc.vector.memset(ones31[:], 1.0)
        ones1x128 = cp.tile([1, 128], F32, name="ones1x128")
        nc.vector.memset(ones1x128[:], 1.0)

        BIG = [128, 4 * N]

        def big(name, tag, bufs=1):
            return wp.tile(BIG, F32, name=name, tag=tag, bufs=bufs)

        def chunk_t(name):
            return wp.tile([128, N], F32, name=name, tag="pch", bufs=4)

        # ---------- emit helpers ----------
        def fft2T_stage1(s, img, tag):
            """stage 1: UT = A^T @ W (psum->sbuf). Returns utr, uti [128,2048]."""
            utr = big(f"utr_{tag}{s}", "ut_r")
            uti = big(f"uti_{tag}{s}", "ut_i")
            for m in range(4):
                pr = pmm.tile([128, N], F32, name=f"p_ut_r{tag}{s}{m}", tag="pmm")
                pi = pmm.tile([128, N], F32, name=f"p_ut_i{tag}{s}{m}", tag="pmm")
                for rc in range(4):
                    lhs = img[:, rc * N + m * 128: rc * N + (m + 1) * 128]
                    nc.tensor.matmul(pr[:], lhs, c["wr"][:, rc * N:(rc + 1) * N],
                                     start=(rc == 0), stop=(rc == 3))
                for rc in range(4):
                    lhs = img[:, rc * N + m * 128: rc * N + (m + 1) * 128]
                    nc.tensor.matmul(pi[:], lhs, c["wi"][:, rc * N:(rc + 1) * N],
                                     start=(rc == 0), stop=(rc == 3))
                nc.scalar.copy(utr[:, m * N:(m + 1) * N], pr[:])
                nc.scalar.copy(uti[:, m * N:(m + 1) * N], pi[:])
            return utr, uti

        def stage2_chunk(prefix, s, mo, utr, uti):
            """stage 2 chunk mo: F^T[mo] in psum (pr, pi)."""
            pr = pmm.tile([128, N], F32, name=f"{prefix}r{s}{mo}", tag="pmm")
            pi = pmm.tile([128, N], F32, name=f"{prefix}i{s}{mo}", tag="pmm")
            for cc in range(4):
                lw = slice(cc * N + mo * 128, cc * N + (mo + 1) * 128)
                nc.tensor.matmul(pr[:], c["wr"][:, lw],
                                 utr[:, cc * N:(cc + 1) * N],
                                 start=(cc == 0), stop=False)
                nc.tensor.matmul(pr[:], c["nwi"][:, lw],
                                 uti[:, cc * N:(cc + 1) * N],
                                 start=False, stop=(cc == 3))
                nc.tensor.matmul(pi[:], c["wr"][:, lw],
                                 uti[:, cc * N:(cc + 1) * N],
                                 start=(cc == 0), stop=False)
                nc.tensor.matmul(pi[:], c["wi"][:, lw],
                                 utr[:, cc * N:(cc + 1) * N],
                                 start=False, stop=(cc == 3))
            return pr, pi

        def crop_ifft(s, gr, gi, lhs_ni, lhs_r2, tag):
            """yp psum [31,31] natural = Re(crop(ifft2(G))) from transposed
            spectrum G (gr, gi [128,2048] sbuf).
            lhs_ni: const/tile for -PlTi (Cr Gi-term); lhs_r2: PlTr for the
            Ci Gi-term (keep-scaled in CG)."""
            crp = ptc.tile([P, N], F32, name=f"crp{tag}{s}", tag="ptc")
            cip = ptc.tile([P, N], F32, name=f"cip{tag}{s}", tag="ptc")
            for cc in range(4):
                ls = slice(cc * P, (cc + 1) * P)
                rs = slice(cc * N, (cc + 1) * N)
                nc.tensor.matmul(crp[:], c["pltr"][:, ls], gr[:, rs],
                                 start=(cc == 0), stop=False)
                nc.tensor.matmul(crp[:], lhs_ni[:, ls], gi[:, rs],
                                 start=False, stop=(cc == 3))
                nc.tensor.matmul(cip[:], lhs_r2[:, ls], gi[:, rs],
                                 start=(cc == 0), stop=False)
                nc.tensor.matmul(cip[:], c["plti"][:, ls], gr[:, rs],
                                 start=False, stop=(cc == 3))
            cr_sb = wp.tile([P, N], F32, name=f"crsb{tag}{s}", tag="csb", bufs=4)
            ci_sb = wp.tile([P, N], F32, name=f"cisb{tag}{s}", tag="csb", bufs=4)
            nc.scalar.copy(cr_sb[:], crp[:])
            nc.scalar.copy(ci_sb[:], cip[:])
            if sub <= 61:
                dbg = wp.tile([P, P], F32, name=f"dbgs61{tag}{s}", tag="junk31", bufs=2)
                nc.vector.tensor_copy(dbg[:], cr_sb[:, :P])
                nc.sync.dma_start(d_out[s], dbg[:])
                return None
            ctp = psml.tile([128, 8 * P], F32, name=f"ctp{tag}{s}", tag="psml")
            for cc in range(4):
                nc.tensor.transpose(ctp[:, cc * P:(cc + 1) * P],
                                    cr_sb[:, cc * 128:(cc + 1) * 128],
                                    c["ident"][:P, :P])
                nc.tensor.transpose(ctp[:, (4 + cc) * P:(5 + cc) * P],
                                    ci_sb[:, cc * 128:(cc + 1) * 128],
                                    c["ident"][:P, :P])
            ct_sb = wp.tile([128, 8 * P], F32, name=f"ctsb{tag}{s}", tag="ctsb",
                            bufs=2)
            nc.scalar.copy(ct_sb[:], ctp[:])
            if sub <= 62:
                dbg = wp.tile([P, P], F32, name=f"dbgs62{tag}{s}", tag="junk31", bufs=2)
                nc.vector.tensor_copy(dbg[:], ct_sb[:P, :P])
                nc.sync.dma_start(d_out[s], dbg[:])
                return None
            yp = psml.tile([P, P], F32, name=f"yp{tag}{s}", tag="psml")
            for cc in range(4):
                nc.tensor.matmul(yp[:], c["prtr"][:, cc * P:(cc + 1) * P],
                                 ct_sb[:, cc * P:(cc + 1) * P],
                                 start=(cc == 0), stop=False)
                nc.tensor.matmul(yp[:], c["nprti"][:, cc * P:(cc + 1) * P],
                                 ct_sb[:, (4 + cc) * P:(5 + cc) * P],
                                 start=False, stop=(cc == 3))
            return yp

        def part_sum_bcast(s, a31, b31, tag):
            """sum(a*b) over [31,31] -> psum [31,1] broadcast on 31 partitions."""
            junk = wp.tile([P, P], F32, name=f"junk{tag}{s}", tag="junk31",
                           bufs=2)
            part = wp.tile([P, 1], F32, name=f"part{tag}{s}", tag="p31", bufs=4)
            nc.vector.tensor_mul(junk[:], a31[:], b31[:])
            nc.vector.tensor_reduce(part[:], junk[:], axis=AX.X, op=ADD)
            sp = psml.tile([P, 1], F32, name=f"sump{tag}{s}", tag="psml")
            nc.tensor.matmul(sp[:], ones31[:], part[:], start=True, stop=True)
            return sp

        # ---------- per-slice state ----------
        lft = [None] * SLICES_PER_CORE
        xs = [None] * SLICES_PER_CORE
        rs_ = [None] * SLICES_PER_CORE
        ps_ = [None] * SLICES_PER_CORE
        rsold = [None] * SLICES_PER_CORE

        # ---------- init phase (per slice; latent first, blur fused) ----------
        for s in range(SLICES_PER_CORE):
            # latent magnitude
            ax_ = big(f"rawlx{s}", "rawA")
            ay_ = big(f"rawly{s}", "rawB")
            nc.sync.dma_start(ax_[:], d_in["lx"][s])
            nc.sync.dma_start(ay_[:], d_in["ly"][s])
            u = big(f"lsqx{s}", "sq1")
            v = big(f"lsqy{s}", "sq2")
            nc.scalar.square(u[:], ax_[:])
            nc.scalar.square(v[:], ay_[:])
            lat = big(f"lat{s}", "img")
            nc.vector.tensor_add(lat[:], u[:], v[:])
            nc.scalar.sqrt(lat[:], lat[:])
            if stage <= 1:
                dbg = wp.tile([P, P], F32, name=f"dbg1_{s}", tag="junk31", bufs=2)
                nc.vector.tensor_copy(dbg[:], lat[:P, :P])
                nc.sync.dma_start(d_out[s], dbg[:])
                continue
            # latent FFT -> fltr, flti in SBUF
            utr, uti = fft2T_stage1(s, lat, "l")
            if stage <= 2:
                dbg = wp.tile([P, P], F32, name=f"dbg2_{s}", tag="junk31", bufs=2)
                nc.vector.tensor_copy(dbg[:], utr[:P, :P])
                nc.sync.dma_start(d_out[s], dbg[:])
                continue
            fltr = big(f"fltr{s}", "fl_r")
            flti = big(f"flti{s}", "fl_i")
            for mo in range(4):
                pr, pi = stage2_chunk("p_fl", s, mo, utr, uti)
                nc.scalar.copy(fltr[:, mo * N:(mo + 1) * N], pr[:])
                nc.scalar.copy(flti[:, mo * N:(mo + 1) * N], pi[:])
            if stage <= 3:
                dbg = wp.tile([P, P], F32, name=f"dbg3_{s}", tag="junk31", bufs=2)
                nc.vector.tensor_copy(dbg[:], fltr[:P, :P])
                nc.sync.dma_start(d_out[s], dbg[:])
                continue
            # lft = fltr^2 + flti^2
            u2 = big(f"lftsq1{s}", "sq1")
            v2 = big(f"lftsq2{s}", "sq2")
            nc.scalar.square(u2[:], fltr[:])
            nc.scalar.square(v2[:], flti[:])
            lft[s] = wp.tile(BIG, F32, name=f"lft{s}", tag=f"lft{s}", bufs=1)
            nc.vector.tensor_add(lft[s][:], u2[:], v2[:])
            if stage <= 4:
                dbg = wp.tile([P, P], F32, name=f"dbg4_{s}", tag="junk31", bufs=2)
                nc.vector.tensor_copy(dbg[:], lft[s][:P, :P])
                nc.sync.dma_start(d_out[s], dbg[:])
                continue
            # blur magnitude
            bx_ = big(f"rawbx{s}", "rawA")
            by_ = big(f"rawby{s}", "rawB")
            nc.sync.dma_start(bx_[:], d_in["bx"][s])
            nc.sync.dma_start(by_[:], d_in["by"][s])
            ub = big(f"bsqx{s}", "sq1")
            vb = big(f"bsqy{s}", "sq2")
            nc.scalar.square(ub[:], bx_[:])
            nc.scalar.square(vb[:], by_[:])
            blur = big(f"blur{s}", "img")
            nc.vector.tensor_add(blur[:], ub[:], vb[:])
            nc.scalar.sqrt(blur[:], blur[:])
            # blur FFT with fused D products (blur spectrum never hits SBUF)
            butr, buti = fft2T_stage1(s, blur, "b")
            dr = big(f"dr_{s}", "dd_r")
            di = big(f"di_{s}", "dd_i")
            for mo in range(4):
                pr, pi = stage2_chunk("p_fb", s, mo, butr, buti)
                rng = slice(mo * N, (mo + 1) * N)
                m1 = chunk_t(f"m1_{s}{mo}")
                m2 = chunk_t(f"m2_{s}{mo}")
                nc.vector.tensor_mul(m1[:], fltr[:, rng], pr[:])
                nc.vector.tensor_mul(m2[:], flti[:, rng], pi[:])
                nc.vector.tensor_add(dr[:, rng], m1[:], m2[:])
                nc.vector.tensor_mul(m1[:], fltr[:, rng], pi[:])
                nc.vector.tensor_mul(m2[:], flti[:, rng], pr[:])
                nc.vector.tensor_sub(di[:, rng], m1[:], m2[:])
                # Dr -= lft * xf0   (xf0 chunk streamed from DRAM)
                xq = chunk_t(f"xq_{s}{mo}")
                nc.sync.dma_start(xq[:], d_c["xf0"][:, rng])
                nc.vector.tensor_mul(xq[:], lft[s][:, rng], xq[:])
                nc.vector.tensor_sub(dr[:, rng], dr[:, rng], xq[:])
            if stage <= 6:
                dbg = wp.tile([P, P], F32, name=f"dbg6_{s}", tag="junk31", bufs=2)
                nc.vector.tensor_copy(dbg[:], dr[:P, :P])
                nc.sync.dma_start(d_out[s], dbg[:])
                continue
            # r0 = cropIFFT(D) - 1/961 ; p0 = r0 ; x0 = 1/961 ; rsold
            yp = crop_ifft(s, dr, di, c["nplti"], c["pltr"], tag="r0")
            if yp is None:
                continue
            if sub <= 63:
                dbg = wp.tile([P, P], F32, name=f"dbgs63{s}", tag="junk31", bufs=2)
                nc.vector.tensor_copy(dbg[:], yp[:])
                nc.sync.dma_start(d_out[s], dbg[:])
                continue
            r0 = wp.tile([P, P], F32, name=f"r_{s}", tag=f"rst{s}", bufs=2)
            nc.vector.tensor_scalar(r0[:], yp[:], -1.0 / (P * P), None, op0=ADD)
            rs_[s] = r0
            p0 = wp.tile([P, P], F32, name=f"p_{s}", tag=f"pst{s}", bufs=2)
            nc.vector.tensor_copy(p0[:], r0[:])
            ps_[s] = p0
            if sub <= 64:
                nc.sync.dma_start(d_out[s], r0[:])
                continue
            x0 = wp.tile([P, P], F32, name=f"x_{s}", tag=f"xst{s}", bufs=2)
            nc.vector.memset(x0[:], 1.0 / (P * P))
            xs[s] = x0
            sp = part_sum_bcast(s, r0, r0, "rs0")
            rso = wp.tile([P, 1], F32, name=f"rsold{s}", tag=f"rso{s}", bufs=2)
            nc.vector.tensor_copy(rso[:], sp[:])
            rsold[s] = rso

        # ---------- CG iterations ----------
        if stage == 7 and rs_[0] is not None:
            for s in range(SLICES_PER_CORE):
                nc.sync.dma_start(d_out[s], rs_[s][:])
        for it in range(n_iter if stage > 7 else 0):
            last = (it == n_iter - 1)
            for s in range(SLICES_PER_CORE):
                p_s = ps_[s]
                # step A: TT = p^T @ WcT
                ttrp = ptc.tile([P, N], F32, name=f"ttrp{s}_{it}", tag="ptc")
                ttip = ptc.tile([P, N], F32, name=f"ttip{s}_{it}", tag="ptc")
                nc.tensor.matmul(ttrp[:], p_s[:], c["wctr"][:], start=True,
                                 stop=True)
                nc.tensor.matmul(ttip[:], p_s[:], c["wcti"][:], start=True,
                                 stop=True)
                ttr = wp.tile([P, N], F32, name=f"ttr{s}_{it}", tag="ttsb",
                              bufs=4)
                tti = wp.tile([P, N], F32, name=f"tti{s}_{it}", tag="ttsb",
                              bufs=4)
                nc.scalar.copy(ttr[:], ttrp[:])
                nc.scalar.copy(tti[:], ttip[:])
                # step B: xf' chunks + mask partials + products
                gr = big(f"gr{s}_{it}", "g_r", bufs=2)
                gi = big(f"gi{s}_{it}", "g_i", bufs=2)
                mip = wp.tile([128, 4], F32, name=f"mip{s}_{it}", tag="mp128",
                              bufs=4)
                mrp = wp.tile([128, 4], F32, name=f"mrp{s}_{it}", tag="mp128",
                              bufs=4)
                for cc in range(4):
                    xrp = pmm.tile([128, N], F32, name=f"xrp{s}_{it}{cc}",
                                   tag="pmm")
                    xip = pmm.tile([128, N], F32, name=f"xip{s}_{it}{cc}",
                                   tag="pmm")
                    lw = slice(cc * 128, (cc + 1) * 128)
                    nc.tensor.matmul(xrp[:], c["wctr"][:, lw], ttr[:],
                                     start=True, stop=False)
                    nc.tensor.matmul(xrp[:], c["nwcti"][:, lw], tti[:],
                                     start=False, stop=True)
                    nc.tensor.matmul(xip[:], c["wcti"][:, lw], ttr[:],
                                     start=True, stop=False)
                    nc.tensor.matmul(xip[:], c["wctr"][:, lw], tti[:],
                                     start=False, stop=True)
                    nc.vector.tensor_reduce(mip[:, cc:cc + 1], xip[:],
                                            axis=AX.X, op=MAX,
                                            apply_absolute_value=True)
                    nc.vector.tensor_reduce(mrp[:, cc:cc + 1], xrp[:],
                                            axis=AX.X, op=MAX,
                                            apply_absolute_value=True)
                    rng = slice(cc * N, (cc + 1) * N)
                    nc.vector.tensor_mul(gr[:, rng], lft[s][:, rng], xrp[:])
                    nc.vector.tensor_mul(gi[:, rng], lft[s][:, rng], xip[:])
                # mask: keep = (mi^2 > t^2*(mi^2+mr^2))
                mi1 = wp.tile([128, 1], F32, name=f"mi1{s}_{it}", tag="k128",
                              bufs=4)
                mr1 = wp.tile([128, 1], F32, name=f"mr1{s}_{it}", tag="k128",
                              bufs=4)
                nc.vector.tensor_reduce(mi1[:], mip[:], axis=AX.X, op=MAX)
                nc.vector.tensor_reduce(mr1[:], mrp[:], axis=AX.X, op=MAX)
                trm = psml.tile([1, 256], F32, name=f"trm{s}_{it}", tag="psml")
                nc.tensor.transpose(trm[:, 0:128], mi1[:], c["ident"][:])
                nc.tensor.transpose(trm[:, 128:256], mr1[:], c["ident"][:])
                mis = wp.tile([1, 4], F32, name=f"mis{s}_{it}", tag="s14",
                              bufs=4)
                nc.vector.tensor_reduce(mis[:, 0:1], trm[:, 0:128], axis=AX.X,
                                        op=MAX)
                nc.vector.tensor_reduce(mis[:, 1:2], trm[:, 128:256],
                                        axis=AX.X, op=MAX)
                nc.vector.tensor_mul(mis[:, 2:3], mis[:, 0:1], mis[:, 0:1])
                nc.vector.tensor_mul(mis[:, 3:4], mis[:, 1:2], mis[:, 1:2])
                keep = wp.tile([1, 2], F32, name=f"keep{s}_{it}", tag="s14",
                               bufs=4)
                nc.vector.tensor_add(keep[:, 1:2], mis[:, 2:3], mis[:, 3:4])
                nc.vector.tensor_scalar(keep[:, 1:2], keep[:, 1:2], T2, None,
                                        op0=MUL)
                nc.vector.tensor_tensor(keep[:, 0:1], mis[:, 2:3],
                                        keep[:, 1:2], op=AluOpType.is_gt)
                kbp = psml.tile([128, 1], F32, name=f"kbp{s}_{it}", tag="psml")
                nc.tensor.matmul(kbp[:], ones1x128[:], keep[:, 0:1],
                                 start=True, stop=True)
                ksb = wp.tile([128, 1], F32, name=f"ksb{s}_{it}", tag="k128",
                              bufs=4)
                nc.vector.tensor_copy(ksb[:], kbp[:])
                pltr_k = wp.tile([128, 4 * P], F32, name=f"pltrk{s}_{it}",
                                 tag="plk", bufs=2)
                nplti_k = wp.tile([128, 4 * P], F32, name=f"npltik{s}_{it}",
                                  tag="nplk", bufs=2)
                nc.vector.tensor_scalar(pltr_k[:], c["pltr"][:], ksb[:], None,
                                        op0=MUL)
                nc.vector.tensor_scalar(nplti_k[:], c["nplti"][:], ksb[:],
                                        None, op0=MUL)
                # steps C/D: Ap = Re(crop(ifft(G))) + p
                yp = crop_ifft(s, gr, gi, nplti_k, pltr_k, tag=f"cg{it}")
                ap_sb = wp.tile([P, P], F32, name=f"ap{s}_{it}", tag="apsb",
                                bufs=2)
                nc.vector.tensor_add(ap_sb[:], yp[:], p_s[:])
                # CG update
                dnp = part_sum_bcast(s, p_s, ap_sb, f"dn{it}")
                alpha = wp.tile([P, 2], F32, name=f"alph{s}_{it}", tag="p31x2",
                                bufs=4)
                nc.vector.reciprocal(alpha[:, 1:2], dnp[:])
                nc.vector.tensor_mul(alpha[:, 0:1], rsold[s][:], alpha[:, 1:2])
                nc.vector.tensor_scalar(alpha[:, 1:2], alpha[:, 0:1], -1.0,
                                        None, op0=MUL)
                xn = wp.tile([P, P], F32, name=f"x_{s}_{it}", tag=f"xst{s}",
                             bufs=2)
                nc.vector.scalar_tensor_tensor(xn[:], p_s[:], alpha[:, 0:1],
                                               xs[s][:], op0=MUL, op1=ADD)
                xs[s] = xn
                if not last:
                    rn = wp.tile([P, P], F32, name=f"r_{s}_{it}",
                                 tag=f"rst{s}", bufs=2)
                    nc.vector.scalar_tensor_tensor(rn[:], ap_sb[:],
                                                   alpha[:, 1:2], rs_[s][:],
                                                   op0=MUL, op1=ADD)
                    rs_[s] = rn
                    rsp = part_sum_bcast(s, rn, rn, f"rs{it}")
                    rsn = wp.tile([P, 1], F32, name=f"rsold{s}_{it}",
                                  tag=f"rso{s}", bufs=2)
                    nc.vector.tensor_copy(rsn[:], rsp[:])
                    beta = wp.tile([P, 2], F32, name=f"beta{s}_{it}",
                                   tag="p31x2", bufs=4)
                    nc.vector.reciprocal(beta[:, 1:2], rsold[s][:])
                    nc.vector.tensor_mul(beta[:, 0:1], rsn[:], beta[:, 1:2])
                    pn = wp.tile([P, P], F32, name=f"p_{s}_{it}",
                                 tag=f"pst{s}", bufs=2)
                    nc.vector.scalar_tensor_tensor(pn[:], p_s[:],
                                                   beta[:, 0:1], rn[:],
                                                   op0=MUL, op1=ADD)
                    ps_[s] = pn
                    rsold[s] = rsn

        # ---------- finalize ----------
        for s in range(SLICES_PER_CORE if stage > 7 else 0):
            x = xs[s]
            xmp = wp.tile([P, 1], F32, name=f"xmp{s}", tag="p31", bufs=4)
            nc.vector.tensor_reduce(xmp[:], x[:], axis=AX.X, op=MAX)
            trx = psml.tile([1, P], F32, name=f"trx{s}", tag="psml")
            nc.tensor.transpose(trx[:], xmp[:], c["ident"][:P, :P])
            mx = wp.tile([1, 1], F32, name=f"mx{s}", tag="s14", bufs=4)
            nc.vector.tensor_reduce(mx[:], trx[:], axis=AX.X, op=MAX)
            nc.vector.tensor_scalar(mx[:], mx[:], 0.05, None, op0=MUL)
            thp = psml.tile([P, 1], F32, name=f"thp{s}", tag="psml")
            nc.tensor.matmul(thp[:], ones31[0:1, :], mx[:], start=True,
                             stop=True)
            thr = wp.tile([P, 1], F32, name=f"thr{s}", tag="p31", bufs=4)
            nc.vector.tensor_copy(thr[:], thp[:])
            km = wp.tile([P, P], F32, name=f"km{s}", tag="junk31", bufs=2)
            nc.vector.tensor_scalar(km[:], x[:], thr[:], None,
                                    op0=AluOpType.is_ge)
            x2 = wp.tile([P, P], F32, name=f"x2_{s}", tag=f"xst{s}", bufs=2)
            nc.vector.tensor_mul(x2[:], x[:], km[:])
            x3 = wp.tile([P, P], F32, name=f"x3_{s}", tag=f"pst{s}", bufs=2)
            nc.vector.tensor_scalar(x3[:], x2[:], 0.0, None, op0=MAX)
            spart = wp.tile([P, 1], F32, name=f"spart{s}", tag="p31", bufs=4)
            nc.vector.tensor_reduce(spart[:], x3[:], axis=AX.X, op=ADD)
            ssp = psml.tile([P, 1], F32, name=f"ssp{s}", tag="psml")
            nc.tensor.matmul(ssp[:], ones31[:], spart[:], start=True,
                             stop=True)
            rcp = wp.tile([P, 1], F32, name=f"rcp{s}", tag="p31", bufs=4)
            nc.vector.reciprocal(rcp[:], ssp[:])
            xo = wp.tile([P, P], F32, name=f"xo{s}", tag=f"rst{s}", bufs=2)
            nc.vector.tensor_scalar(xo[:], x3[:], rcp[:], None, op0=MUL)
            nc.sync.dma_start(d_out[s], xo[:])

    nc.compile()
    return nc


def _get_program(n_iter=N_ITER):
    key = ("nc", n_iter)
    if key not in _PROGRAM_CACHE:
        _PROGRAM_CACHE[key] = _build_program(n_iter)
    return _PROGRAM_CACHE[key]


def _core_assignment(b, cch):
    pairs = [(bi, ci) for bi in range(b) for ci in range(cch)]
    ext = list(pairs)
    while len(ext) < NCORES * SLICES_PER_CORE:
        ext.append(pairs[len(ext) - len(pairs)])
    return [(ext[k], ext[k + NCORES]) for k in range(NCORES)]


def kernel(blurx, blury, latentx, latenty, psf_size):
    psf_size = int(np.asarray(psf_size))
    assert psf_size == P, f"kernel hardcoded for psf_size=31, got {psf_size}"
    blurx = np.asarray(blurx, dtype=np.float32)
    blury = np.asarray(blury, dtype=np.float32)
    latentx = np.asarray(latentx, dtype=np.float32)
    latenty = np.asarray(latenty, dtype=np.float32)
    b, cch, H, W = blurx.shape
    assert (H, W) == (N, N)
    nc = _get_program()
    consts = _make_consts()
    percore = _core_assignment(b, cch)
    in_maps = []
    for k in range(NCORES):
        m = dict(consts)
        for nm, arr in (("bx", blurx), ("by", blury),
                        ("lx", latentx), ("ly", latenty)):
            m[nm] = np.stack([_to_sb(arr[bi, ci]) for (bi, ci) in percore[k]])
        in_maps.append(m)
    from concourse.bass_utils import run_bass_kernel_spmd
    res = run_bass_kernel_spmd(nc, in_maps, core_ids=list(range(NCORES)))
    out = np.zeros((b, cch, P, P), np.float32)
    done = set()
    for k in range(NCORES):
        for j, (bi, ci) in enumerate(percore[k]):
            if (bi, ci) not in done:
                out[bi, ci] = res.results[k]["out"][j]
                done.add((bi, ci))
    return out


if __name__ == "__main__":
    d = np.load('/root/problem/ref_inputs.npz')
    out = kernel(d['blurx'], d['blury'], d['latentx'], d['latenty'], 31)
    ref = np.load('/root/problem/ref_out.npy')
    err = np.abs(out - ref)
    print("absmax rel:", err.max() / np.abs(ref).max())
    print("fro rel:", np.linalg.norm(out - ref) / np.linalg.norm(ref))


# revision 18
# speedup vs baseline: 2.6893x; 2.6893x over previous
"""Trainium2 Bass kernel for nn_EstimatePSF: FFT-based PSF estimation via CG.

Strategy:
- All 2D FFTs/IFFTs expressed as DFT matmuls on the TensorEngine (fp32).
  Rolls/pads/crops are absorbed into precomputed DFT-matrix constants.
- Data-parallel over the 12 (b,c) slices; SPMD over 8 cores, 2 slices per
  core (4 slices duplicated to fill 16 = 8*2 program slots). No collectives.
- All 512x512 spectra live TRANSPOSED ("spectrum layout"); the 31x31 CG
  state stays natural. crop-IFFT swaps lhsT/rhs in its last stage so the
  natural orientation comes back for free.
- r0 computed via linearity: D = bf - lft*xf0 (xf0 = analytic spectrum of
  the uniform init kernel, masked -> real), r0 = cropIFFT(D) - x0.
- The psf2otf imag-mask is computed with max|Im|, max|Re| reductions and
  applied by scaling the imag-term DFT constants by keep (0/1) - exact.

Self-contained: hardcodes shapes (4,3,512,512) f32, psf_size=31.
"""
import sys
import math as _math
import numpy as np

sys.path.insert(0, '/opt/trn_rl_repo')

P = 31
N = 512
EPS32 = 1.1920928955078125e-07
NOPS_T = np.float32(P * P * (2.0 * _math.log2(P)) * EPS32)
T2 = float(np.float32(np.float32(NOPS_T) * np.float32(NOPS_T)))
N_ITER = 10
NCORES = 8
SLICES_PER_CORE = 2


def _to_sb(a):
    """[512, X] row-major -> SBUF layout [128, 4X] (4 row-chunks side by side)."""
    X = a.shape[1]
    return np.ascontiguousarray(
        a.reshape(4, 128, X).transpose(1, 0, 2).reshape(128, 4 * X))


def _make_consts():
    k = np.arange(N)
    ang = -2.0 * np.pi * np.outer(k, k) / N
    Wr = np.cos(ang).astype(np.float32)   # symmetric
    Wi = np.sin(ang).astype(np.float32)
    i31 = np.arange(P) - (P // 2)
    angc = -2.0 * np.pi * np.outer(k, i31) / N   # [512, 31] : Wc
    WcTr = np.cos(angc).astype(np.float32).T.copy()  # [31, 512]
    WcTi = np.sin(angc).astype(np.float32).T.copy()
    angp = 2.0 * np.pi * np.outer(i31, k) / N    # [31, 512]
    Er = np.cos(angp).astype(np.float64)
    Ei = np.sin(angp).astype(np.float64)
    PlTr = (Er / (N * N)).astype(np.float32).T.copy()  # [512, 31]
    PlTi = (Ei / (N * N)).astype(np.float32).T.copy()
    PrTr = Er.astype(np.float32).T.copy()
    PrTi = Ei.astype(np.float32).T.copy()
    with np.errstate(invalid='ignore', divide='ignore'):
        D31 = np.sin(31 * np.pi * k / N) / np.sin(np.pi * k / N)
    D31[0] = 31.0
    xf0 = (np.outer(D31, D31) / (P * P)).astype(np.float32)
    # stacked 2-band constants for PE row-packing (band0 rows 0-30,
    # band1 rows 32-62; row 31/63 zero-padding)
    def stack2(a, b):
        out = np.zeros((63, a.shape[1]), np.float32)
        out[0:31] = a
        out[32:63] = b
        return out
    wcpa = stack2(WcTr, WcTi)            # xf pair-1 lhsT (rhs = ttr both bands)
    wcpb = stack2(-WcTi, WcTr)           # xf pair-2 lhsT (rhs = tti both bands)
    wcts = stack2(WcTr, WcTi)            # TT-step rhs stack (lhsT = p both bands)
    ident2 = stack2(np.eye(P, dtype=np.float32), np.eye(P, dtype=np.float32))
    return {
        "wr": _to_sb(Wr), "wi": _to_sb(Wi), "nwi": _to_sb(-Wi),
        "wcpa": wcpa, "wcpb": wcpb, "wcts": wcts, "ident2": ident2,
        "pltr": _to_sb(PlTr), "plti": _to_sb(PlTi), "nplti": _to_sb(-PlTi),
        "prtr": _to_sb(PrTr), "nprti": _to_sb(-PrTi),
        "xf0": _to_sb(xf0),
        "ident": np.eye(128, dtype=np.float32),
    }


_PROGRAM_CACHE = {}


def _build_program(n_iter=N_ITER, stage=99, sub=99):
    from contextlib import ExitStack
    import concourse.bacc as bacc
    import concourse.tile as tile
    from concourse import mybir
    from concourse.alu_op_type import AluOpType

    F32 = mybir.dt.float32
    AX = mybir.AxisListType
    MUL = AluOpType.mult
    ADD = AluOpType.add
    MAX = AluOpType.max

    nc = bacc.Bacc(None, target_bir_lowering=False, debug=False)

    # ---- DRAM ----
    d_in = {}
    for nm in ("bx", "by", "lx", "ly"):
        d_in[nm] = nc.dram_tensor(nm, [SLICES_PER_CORE, 128, 4 * N], F32,
                                  kind="ExternalInput").ap()
    d_c = {}
    for nm, shp in (("wr", [128, 4 * N]), ("wi", [128, 4 * N]),
                    ("nwi", [128, 4 * N]),
                    ("wcpa", [63, N]), ("wcpb", [63, N]), ("wcts", [63, N]),
                    ("ident2", [63, P]),
                    ("pltr", [128, 4 * P]), ("plti", [128, 4 * P]),
                    ("nplti", [128, 4 * P]),
                    ("prtr", [128, 4 * P]), ("nprti", [128, 4 * P]),
                    ("xf0", [128, 4 * N]), ("ident", [128, 128])):
        d_c[nm] = nc.dram_tensor(nm, shp, F32, kind="ExternalInput").ap()
    d_out = nc.dram_tensor("out", [SLICES_PER_CORE, P, P], F32,
                           kind="ExternalOutput").ap()

    with tile.TileContext(nc) as tc, ExitStack() as ctx:
        cp = ctx.enter_context(tc.tile_pool(name="consts", bufs=1))
        wp = ctx.enter_context(tc.tile_pool(name="work", bufs=1))
        pmm = ctx.enter_context(tc.tile_pool(name="pmm", bufs=4, space="PSUM"))
        ptc = ctx.enter_context(tc.tile_pool(name="ptc", bufs=2, space="PSUM"))
        psml = ctx.enter_context(tc.tile_pool(name="psml", bufs=2, space="PSUM"))

        # ---- constants to SBUF ----
        c = {}
        for nm in d_c:
            if nm == "xf0":
                continue  # streamed chunk-wise from DRAM
            c[nm] = cp.tile(list(d_c[nm].shape), F32, name=f"c_{nm}")
            nc.sync.dma_start(c[nm][:], d_c[nm][:])
        ones31 = cp.tile([P, P], F32, name="ones31")
        nc.vector.memset(ones31[:], 1.0)
        ones1x128 = cp.tile([1, 128], F32, name="ones1x128")
        nc.vector.memset(ones1x128[:], 1.0)

        BIG = [128, 4 * N]

        def big(name, tag, bufs=1):
            return wp.tile(BIG, F32, name=name, tag=tag, bufs=bufs)

        def chunk_t(name):
            return wp.tile([128, N], F32, name=name, tag="pch", bufs=4)

        # ---------- emit helpers ----------
        def fft2T_stage1(s, img, tag):
            """stage 1: UT = A^T @ W (psum->sbuf). Returns utr, uti [128,2048]."""
            utr = big(f"utr_{tag}{s}", "ut_r")
            uti = big(f"uti_{tag}{s}", "ut_i")
            for m in range(4):
                pr = pmm.tile([128, N], F32, name=f"p_ut_r{tag}{s}{m}", tag="pmm")
                pi = pmm.tile([128, N], F32, name=f"p_ut_i{tag}{s}{m}", tag="pmm")
                for rc in range(4):
                    lhs = img[:, rc * N + m * 128: rc * N + (m + 1) * 128]
                    nc.tensor.matmul(pr[:], lhs, c["wr"][:, rc * N:(rc + 1) * N],
                                     start=(rc == 0), stop=(rc == 3))
                for rc in range(4):
                    lhs = img[:, rc * N + m * 128: rc * N + (m + 1) * 128]
                    nc.tensor.matmul(pi[:], lhs, c["wi"][:, rc * N:(rc + 1) * N],
                                     start=(rc == 0), stop=(rc == 3))
                nc.scalar.copy(utr[:, m * N:(m + 1) * N], pr[:])
                nc.scalar.copy(uti[:, m * N:(m + 1) * N], pi[:])
            return utr, uti

        def stage2_chunk(prefix, s, mo, utr, uti):
            """stage 2 chunk mo: F^T[mo] in psum (pr, pi)."""
            pr = pmm.tile([128, N], F32, name=f"{prefix}r{s}{mo}", tag="pmm")
            pi = pmm.tile([128, N], F32, name=f"{prefix}i{s}{mo}", tag="pmm")
            for cc in range(4):
                lw = slice(cc * N + mo * 128, cc * N + (mo + 1) * 128)
                nc.tensor.matmul(pr[:], c["wr"][:, lw],
                                 utr[:, cc * N:(cc + 1) * N],
                                 start=(cc == 0), stop=False)
                nc.tensor.matmul(pr[:], c["nwi"][:, lw],
                                 uti[:, cc * N:(cc + 1) * N],
                                 start=False, stop=(cc == 3))
                nc.tensor.matmul(pi[:], c["wr"][:, lw],
                                 uti[:, cc * N:(cc + 1) * N],
                                 start=(cc == 0), stop=False)
                nc.tensor.matmul(pi[:], c["wi"][:, lw],
                                 utr[:, cc * N:(cc + 1) * N],
                                 start=False, stop=(cc == 3))
            return pr, pi

        def crop_ifft(s, gr, gi, lhs_ni, lhs_r2, tag):
            """yp psum [31,31] natural = Re(crop(ifft2(G))) from transposed
            spectrum G (gr, gi [128,2048] sbuf).
            lhs_ni: const/tile for -PlTi (Cr Gi-term); lhs_r2: PlTr for the
            Ci Gi-term (keep-scaled in CG)."""
            # col-packed: Cr accumulates in rows 0-30 (array cols 0-31),
            # Ci in rows 32-62 (array cols 32-63); pairs share the rhs stream.
            cpk = ptc.tile([63, N], F32, name=f"cpk{tag}{s}", tag="ptc")
            for cc in range(4):
                ls = slice(cc * P, (cc + 1) * P)
                rs = slice(cc * N, (cc + 1) * N)
                nc.tensor.matmul(cpk[0:31, :], c["pltr"][:, ls], gr[:, rs],
                                 start=(cc == 0), stop=False,
                                 tile_position=(0, 0), skip_group_check=True)
                nc.tensor.matmul(cpk[32:63, :], c["plti"][:, ls], gr[:, rs],
                                 start=(cc == 0), stop=False,
                                 tile_position=(0, 32), skip_group_check=True)
                nc.tensor.matmul(cpk[0:31, :], lhs_ni[:, ls], gi[:, rs],
                                 start=False, stop=(cc == 3),
                                 tile_position=(0, 0), skip_group_check=True)
                nc.tensor.matmul(cpk[32:63, :], lhs_r2[:, ls], gi[:, rs],
                                 start=False, stop=(cc == 3),
                                 tile_position=(0, 32), skip_group_check=True)
            cr_sb = wp.tile([P, N], F32, name=f"crsb{tag}{s}", tag="csb",
                            bufs=4)
            ci_sb = wp.tile([P, N], F32, name=f"cisb{tag}{s}", tag="csb",
                            bufs=4)
            nc.scalar.copy(cr_sb[:], cpk[0:31, :])
            nc.scalar.copy(ci_sb[:], cpk[32:63, :])
            if sub <= 61:
                dbg = wp.tile([P, P], F32, name=f"dbgs61{tag}{s}", tag="junk31", bufs=2)
                nc.vector.tensor_copy(dbg[:], cr_sb[:, :P])
                nc.sync.dma_start(d_out[s], dbg[:])
                return None
            ctp = psml.tile([128, 8 * P], F32, name=f"ctp{tag}{s}", tag="psml")
            for cc in range(4):
                nc.tensor.transpose(ctp[:, cc * P:(cc + 1) * P],
                                    cr_sb[:, cc * 128:(cc + 1) * 128],
                                    c["ident"][:P, :P])
                nc.tensor.transpose(ctp[:, (4 + cc) * P:(5 + cc) * P],
                                    ci_sb[:, cc * 128:(cc + 1) * 128],
                                    c["ident"][:P, :P])
            ct_sb = wp.tile([128, 8 * P], F32, name=f"ctsb{tag}{s}", tag="ctsb",
                            bufs=2)
            nc.scalar.copy(ct_sb[:], ctp[:])
            if sub <= 62:
                dbg = wp.tile([P, P], F32, name=f"dbgs62{tag}{s}", tag="junk31", bufs=2)
                nc.vector.tensor_copy(dbg[:], ct_sb[:P, :P])
                nc.sync.dma_start(d_out[s], dbg[:])
                return None
            yp = psml.tile([P, P], F32, name=f"yp{tag}{s}", tag="psml")
            for cc in range(4):
                nc.tensor.matmul(yp[:], c["prtr"][:, cc * P:(cc + 1) * P],
                                 ct_sb[:, cc * P:(cc + 1) * P],
                                 start=(cc == 0), stop=False)
                nc.tensor.matmul(yp[:], c["nprti"][:, cc * P:(cc + 1) * P],
                                 ct_sb[:, (4 + cc) * P:(5 + cc) * P],
                                 start=False, stop=(cc == 3))
            return yp

        def part_sum_bcast(s, a31, b31, tag):
            """sum(a*b) over [31,31] -> psum [31,1] broadcast on 31 partitions."""
            junk = wp.tile([P, P], F32, name=f"junk{tag}{s}", tag="junk31",
                           bufs=2)
            part = wp.tile([P, 1], F32, name=f"part{tag}{s}", tag="p31", bufs=4)
            nc.vector.tensor_mul(junk[:], a31[:], b31[:])
            nc.vector.tensor_reduce(part[:], junk[:], axis=AX.X, op=ADD)
            sp = psml.tile([P, 1], F32, name=f"sump{tag}{s}", tag="psml")
            nc.tensor.matmul(sp[:], ones31[:], part[:], start=True, stop=True)
            return sp

        # ---------- per-slice state ----------
        lft = [None] * SLICES_PER_CORE
        xs = [None] * SLICES_PER_CORE
        rs_ = [None] * SLICES_PER_CORE
        ps_ = [None] * SLICES_PER_CORE
        rsold = [None] * SLICES_PER_CORE

        # ---------- init phase (per slice; latent first, blur fused) ----------
        for s in range(SLICES_PER_CORE):
            # latent magnitude
            ax_ = big(f"rawlx{s}", "rawA")
            ay_ = big(f"rawly{s}", "rawB")
            nc.sync.dma_start(ax_[:], d_in["lx"][s])
            nc.sync.dma_start(ay_[:], d_in["ly"][s])
            u = big(f"lsqx{s}", "sq1")
            v = big(f"lsqy{s}", "sq2")
            nc.scalar.square(u[:], ax_[:])
            nc.scalar.square(v[:], ay_[:])
            lat = big(f"lat{s}", "img")
            nc.vector.tensor_add(lat[:], u[:], v[:])
            nc.scalar.sqrt(lat[:], lat[:])
            if stage <= 1:
                dbg = wp.tile([P, P], F32, name=f"dbg1_{s}", tag="junk31", bufs=2)
                nc.vector.tensor_copy(dbg[:], lat[:P, :P])
                nc.sync.dma_start(d_out[s], dbg[:])
                continue
            # latent FFT -> fltr, flti in SBUF
            utr, uti = fft2T_stage1(s, lat, "l")
            if stage <= 2:
                dbg = wp.tile([P, P], F32, name=f"dbg2_{s}", tag="junk31", bufs=2)
                nc.vector.tensor_copy(dbg[:], utr[:P, :P])
                nc.sync.dma_start(d_out[s], dbg[:])
                continue
            fltr = big(f"fltr{s}", "fl_r")
            flti = big(f"flti{s}", "fl_i")
            for mo in range(4):
                pr, pi = stage2_chunk("p_fl", s, mo, utr, uti)
                nc.scalar.copy(fltr[:, mo * N:(mo + 1) * N], pr[:])
                nc.scalar.copy(flti[:, mo * N:(mo + 1) * N], pi[:])
            if stage <= 3:
                dbg = wp.tile([P, P], F32, name=f"dbg3_{s}", tag="junk31", bufs=2)
                nc.vector.tensor_copy(dbg[:], fltr[:P, :P])
                nc.sync.dma_start(d_out[s], dbg[:])
                continue
            # lft = fltr^2 + flti^2
            u2 = big(f"lftsq1{s}", "sq1")
            v2 = big(f"lftsq2{s}", "sq2")
            nc.scalar.square(u2[:], fltr[:])
            nc.scalar.square(v2[:], flti[:])
            lft[s] = wp.tile(BIG, F32, name=f"lft{s}", tag=f"lft{s}", bufs=1)
            nc.vector.tensor_add(lft[s][:], u2[:], v2[:])
            if stage <= 4:
                dbg = wp.tile([P, P], F32, name=f"dbg4_{s}", tag="junk31", bufs=2)
                nc.vector.tensor_copy(dbg[:], lft[s][:P, :P])
                nc.sync.dma_start(d_out[s], dbg[:])
                continue
            # blur magnitude
            bx_ = big(f"rawbx{s}", "rawA")
            by_ = big(f"rawby{s}", "rawB")
            nc.sync.dma_start(bx_[:], d_in["bx"][s])
            nc.sync.dma_start(by_[:], d_in["by"][s])
            ub = big(f"bsqx{s}", "sq1")
            vb = big(f"bsqy{s}", "sq2")
            nc.scalar.square(ub[:], bx_[:])
            nc.scalar.square(vb[:], by_[:])
            blur = big(f"blur{s}", "img")
            nc.vector.tensor_add(blur[:], ub[:], vb[:])
            nc.scalar.sqrt(blur[:], blur[:])
            # blur FFT with fused D products (blur spectrum never hits SBUF)
            butr, buti = fft2T_stage1(s, blur, "b")
            dr = big(f"dr_{s}", "dd_r")
            di = big(f"di_{s}", "dd_i")
            for mo in range(4):
                pr, pi = stage2_chunk("p_fb", s, mo, butr, buti)
                rng = slice(mo * N, (mo + 1) * N)
                m1 = chunk_t(f"m1_{s}{mo}")
                m2 = chunk_t(f"m2_{s}{mo}")
                nc.vector.tensor_mul(m1[:], fltr[:, rng], pr[:])
                nc.vector.tensor_mul(m2[:], flti[:, rng], pi[:])
                nc.vector.tensor_add(dr[:, rng], m1[:], m2[:])
                nc.vector.tensor_mul(m1[:], fltr[:, rng], pi[:])
                nc.vector.tensor_mul(m2[:], flti[:, rng], pr[:])
                nc.vector.tensor_sub(di[:, rng], m1[:], m2[:])
                # Dr -= lft * xf0   (xf0 chunk streamed from DRAM)
                xq = chunk_t(f"xq_{s}{mo}")
                nc.sync.dma_start(xq[:], d_c["xf0"][:, rng])
                nc.vector.tensor_mul(xq[:], lft[s][:, rng], xq[:])
                nc.vector.tensor_sub(dr[:, rng], dr[:, rng], xq[:])
            if stage <= 6:
                dbg = wp.tile([P, P], F32, name=f"dbg6_{s}", tag="junk31", bufs=2)
                nc.vector.tensor_copy(dbg[:], dr[:P, :P])
                nc.sync.dma_start(d_out[s], dbg[:])
                continue
            # r0 = cropIFFT(D) - 1/961 ; p0 = r0 ; x0 = 1/961 ; rsold
            yp = crop_ifft(s, dr, di, c["nplti"], c["pltr"], tag="r0")
            if yp is None:
                continue
            if sub <= 63:
                dbg = wp.tile([P, P], F32, name=f"dbgs63{s}", tag="junk31", bufs=2)
                nc.vector.tensor_copy(dbg[:], yp[:])
                nc.sync.dma_start(d_out[s], dbg[:])
                continue
            r0 = wp.tile([P, P], F32, name=f"r_{s}", tag=f"rst{s}", bufs=2)
            nc.vector.tensor_scalar(r0[:], yp[:], -1.0 / (P * P), None, op0=ADD)
            rs_[s] = r0
            p0 = wp.tile([P, P], F32, name=f"p_{s}", tag=f"pst{s}", bufs=2)
            nc.vector.tensor_copy(p0[:], r0[:])
            ps_[s] = p0
            if sub <= 64:
                nc.sync.dma_start(d_out[s], r0[:])
                continue
            x0 = wp.tile([P, P], F32, name=f"x_{s}", tag=f"xst{s}", bufs=2)
            nc.vector.memset(x0[:], 1.0 / (P * P))
            xs[s] = x0
            sp = part_sum_bcast(s, r0, r0, "rs0")
            rso = wp.tile([P, 1], F32, name=f"rsold{s}", tag=f"rso{s}", bufs=2)
            nc.vector.tensor_copy(rso[:], sp[:])
            rsold[s] = rso

        # ---------- CG iterations ----------
        if stage == 7 and rs_[0] is not None:
            for s in range(SLICES_PER_CORE):
                nc.sync.dma_start(d_out[s], rs_[s][:])
        for it in range(n_iter if stage > 7 else 0):
            last = (it == n_iter - 1)
            for s in range(SLICES_PER_CORE):
                p_s = ps_[s]
                # step A (row-packed pair): TTr = p^T@WcTr (band0),
                # TTi = p^T@WcTi (band1). lhsT = p stacked at both bands.
                pstk = wp.tile([63, P], F32, name=f"pstk{s}_{it}", tag="pstk",
                               bufs=4)
                nc.scalar.copy(pstk[0:31, :], p_s[:])
                nc.scalar.copy(pstk[32:63, :], p_s[:])
                ttrp = ptc.tile([P, N], F32, name=f"ttrp{s}_{it}", tag="ptc")
                ttip = ptc.tile([P, N], F32, name=f"ttip{s}_{it}", tag="ptc")
                nc.tensor.matmul(ttrp[:], pstk[0:31, :], c["wcts"][0:31, :],
                                 start=True, stop=True, tile_position=(0, 0))
                nc.tensor.matmul(ttip[:], pstk[32:63, :], c["wcts"][32:63, :],
                                 start=True, stop=True, tile_position=(32, 0))
                # TT to SBUF, stacked twice for the row-packed xf step:
                # tt_rr = [ttr; ttr], tt_ii = [tti; tti]
                tt_rr = wp.tile([63, N], F32, name=f"ttrr{s}_{it}", tag="ttsb",
                                bufs=4)
                tt_ii = wp.tile([63, N], F32, name=f"ttii{s}_{it}", tag="ttsb",
                                bufs=4)
                nc.scalar.copy(tt_rr[0:31, :], ttrp[:])
                nc.scalar.copy(tt_rr[32:63, :], ttrp[:])
                nc.scalar.copy(tt_ii[0:31, :], ttip[:])
                nc.scalar.copy(tt_ii[32:63, :], ttip[:])
                # step B: xf' chunks + mask partials + products
                gr = big(f"gr{s}_{it}", "g_r", bufs=2)
                gi = big(f"gi{s}_{it}", "g_i", bufs=2)
                mip = wp.tile([128, 4], F32, name=f"mip{s}_{it}", tag="mp128",
                              bufs=4)
                mrp = wp.tile([128, 4], F32, name=f"mrp{s}_{it}", tag="mp128",
                              bufs=4)
                for cc in range(4):
                    xrp = pmm.tile([128, N], F32, name=f"xrp{s}_{it}{cc}",
                                   tag="pmm")
                    xip = pmm.tile([128, N], F32, name=f"xip{s}_{it}{cc}",
                                   tag="pmm")
                    lw = slice(cc * 128, (cc + 1) * 128)
                    # row-packed pairs: (xr+=WcTr@ttr | xi+=WcTi@ttr) then
                    # (xr+=-WcTi@tti | xi+=WcTr@tti); banks differ per pair.
                    nc.tensor.matmul(xrp[:], c["wcpa"][0:31, lw],
                                     tt_rr[0:31, :], start=True, stop=False,
                                     tile_position=(0, 0))
                    nc.tensor.matmul(xip[:], c["wcpa"][32:63, lw],
                                     tt_rr[32:63, :], start=True, stop=False,
                                     tile_position=(32, 0))
                    nc.tensor.matmul(xrp[:], c["wcpb"][0:31, lw],
                                     tt_ii[0:31, :], start=False, stop=True,
                                     tile_position=(0, 0))
                    nc.tensor.matmul(xip[:], c["wcpb"][32:63, lw],
                                     tt_ii[32:63, :], start=False, stop=True,
                                     tile_position=(32, 0))
                    nc.vector.tensor_reduce(mip[:, cc:cc + 1], xip[:],
                                            axis=AX.X, op=MAX,
                                            apply_absolute_value=True)
                    nc.vector.tensor_reduce(mrp[:, cc:cc + 1], xrp[:],
                                            axis=AX.X, op=MAX,
                                            apply_absolute_value=True)
                    rng = slice(cc * N, (cc + 1) * N)
                    nc.vector.tensor_mul(gr[:, rng], lft[s][:, rng], xrp[:])
                    nc.vector.tensor_mul(gi[:, rng], lft[s][:, rng], xip[:])
                # mask: keep = (mi^2 > t^2*(mi^2+mr^2))
                mi1 = wp.tile([128, 1], F32, name=f"mi1{s}_{it}", tag="k128",
                              bufs=4)
                mr1 = wp.tile([128, 1], F32, name=f"mr1{s}_{it}", tag="k128",
                              bufs=4)
                nc.vector.tensor_reduce(mi1[:], mip[:], axis=AX.X, op=MAX)
                nc.vector.tensor_reduce(mr1[:], mrp[:], axis=AX.X, op=MAX)
                trm = psml.tile([1, 256], F32, name=f"trm{s}_{it}", tag="psml")
                nc.tensor.transpose(trm[:, 0:128], mi1[:], c["ident"][:])
                nc.tensor.transpose(trm[:, 128:256], mr1[:], c["ident"][:])
                mis = wp.tile([1, 4], F32, name=f"mis{s}_{it}", tag="s14",
                              bufs=4)
                nc.vector.tensor_reduce(mis[:, 0:1], trm[:, 0:128], axis=AX.X,
                                        op=MAX)
                nc.vector.tensor_reduce(mis[:, 1:2], trm[:, 128:256],
                                        axis=AX.X, op=MAX)
                nc.vector.tensor_mul(mis[:, 2:3], mis[:, 0:1], mis[:, 0:1])
                nc.vector.tensor_mul(mis[:, 3:4], mis[:, 1:2], mis[:, 1:2])
                keep = wp.tile([1, 2], F32, name=f"keep{s}_{it}", tag="s14",
                               bufs=4)
                nc.vector.tensor_add(keep[:, 1:2], mis[:, 2:3], mis[:, 3:4])
                nc.vector.tensor_scalar(keep[:, 1:2], keep[:, 1:2], T2, None,
                                        op0=MUL)
                nc.vector.tensor_tensor(keep[:, 0:1], mis[:, 2:3],
                                        keep[:, 1:2], op=AluOpType.is_gt)
                kbp = psml.tile([128, 1], F32, name=f"kbp{s}_{it}", tag="psml")
                nc.tensor.matmul(kbp[:], ones1x128[:], keep[:, 0:1],
                                 start=True, stop=True)
                ksb = wp.tile([128, 1], F32, name=f"ksb{s}_{it}", tag="k128",
                              bufs=4)
                nc.vector.tensor_copy(ksb[:], kbp[:])
                pltr_k = wp.tile([128, 4 * P], F32, name=f"pltrk{s}_{it}",
                                 tag="plk", bufs=2)
                nplti_k = wp.tile([128, 4 * P], F32, name=f"npltik{s}_{it}",
                                  tag="nplk", bufs=2)
                nc.vector.tensor_scalar(pltr_k[:], c["pltr"][:], ksb[:], None,
                                        op0=MUL)
                nc.vector.tensor_scalar(nplti_k[:], c["nplti"][:], ksb[:],
                                        None, op0=MUL)
                # steps C/D: Ap = Re(crop(ifft(G))) + p
                yp = crop_ifft(s, gr, gi, nplti_k, pltr_k, tag=f"cg{it}")
                ap_sb = wp.tile([P, P], F32, name=f"ap{s}_{it}", tag="apsb",
                                bufs=2)
                nc.vector.tensor_add(ap_sb[:], yp[:], p_s[:])
                # CG update
                dnp = part_sum_bcast(s, p_s, ap_sb, f"dn{it}")
                alpha = wp.tile([P, 2], F32, name=f"alph{s}_{it}", tag="p31x2",
                                bufs=4)
                nc.vector.reciprocal(alpha[:, 1:2], dnp[:])
                nc.vector.tensor_mul(alpha[:, 0:1], rsold[s][:], alpha[:, 1:2])
                nc.vector.tensor_scalar(alpha[:, 1:2], alpha[:, 0:1], -1.0,
                                        None, op0=MUL)
                xn = wp.tile([P, P], F32, name=f"x_{s}_{it}", tag=f"xst{s}",
                             bufs=2)
                nc.vector.scalar_tensor_tensor(xn[:], p_s[:], alpha[:, 0:1],
                                               xs[s][:], op0=MUL, op1=ADD)
                xs[s] = xn
                if not last:
                    rn = wp.tile([P, P], F32, name=f"r_{s}_{it}",
                                 tag=f"rst{s}", bufs=2)
                    nc.vector.scalar_tensor_tensor(rn[:], ap_sb[:],
                                                   alpha[:, 1:2], rs_[s][:],
                                                   op0=MUL, op1=ADD)
                    rs_[s] = rn
                    rsp = part_sum_bcast(s, rn, rn, f"rs{it}")
                    rsn = wp.tile([P, 1], F32, name=f"rsold{s}_{it}",
                                  tag=f"rso{s}", bufs=2)
                    nc.vector.tensor_copy(rsn[:], rsp[:])
                    beta = wp.tile([P, 2], F32, name=f"beta{s}_{it}",
                                   tag="p31x2", bufs=4)
                    nc.vector.reciprocal(beta[:, 1:2], rsold[s][:])
                    nc.vector.tensor_mul(beta[:, 0:1], rsn[:], beta[:, 1:2])
                    pn = wp.tile([P, P], F32, name=f"p_{s}_{it}",
                                 tag=f"pst{s}", bufs=2)
                    nc.vector.scalar_tensor_tensor(pn[:], p_s[:],
                                                   beta[:, 0:1], rn[:],
                                                   op0=MUL, op1=ADD)
                    ps_[s] = pn
                    rsold[s] = rsn

        # ---------- finalize ----------
        for s in range(SLICES_PER_CORE if stage > 7 else 0):
            x = xs[s]
            xmp = wp.tile([P, 1], F32, name=f"xmp{s}", tag="p31", bufs=4)
            nc.vector.tensor_reduce(xmp[:], x[:], axis=AX.X, op=MAX)
            trx = psml.tile([1, P], F32, name=f"trx{s}", tag="psml")
            nc.tensor.transpose(trx[:], xmp[:], c["ident"][:P, :P])
            mx = wp.tile([1, 1], F32, name=f"mx{s}", tag="s14", bufs=4)
            nc.vector.tensor_reduce(mx[:], trx[:], axis=AX.X, op=MAX)
            nc.vector.tensor_scalar(mx[:], mx[:], 0.05, None, op0=MUL)
            thp = psml.tile([P, 1], F32, name=f"thp{s}", tag="psml")
            nc.tensor.matmul(thp[:], ones31[0:1, :], mx[:], start=True,
                             stop=True)
            thr = wp.tile([P, 1], F32, name=f"thr{s}", tag="p31", bufs=4)
            nc.vector.tensor_copy(thr[:], thp[:])
            km = wp.tile([P, P], F32, name=f"km{s}", tag="junk31", bufs=2)
            nc.vector.tensor_scalar(km[:], x[:], thr[:], None,
                                    op0=AluOpType.is_ge)
            x2 = wp.tile([P, P], F32, name=f"x2_{s}", tag=f"xst{s}", bufs=2)
            nc.vector.tensor_mul(x2[:], x[:], km[:])
            x3 = wp.tile([P, P], F32, name=f"x3_{s}", tag=f"pst{s}", bufs=2)
            nc.vector.tensor_scalar(x3[:], x2[:], 0.0, None, op0=MAX)
            spart = wp.tile([P, 1], F32, name=f"spart{s}", tag="p31", bufs=4)
            nc.vector.tensor_reduce(spart[:], x3[:], axis=AX.X, op=ADD)
            ssp = psml.tile([P, 1], F32, name=f"ssp{s}", tag="psml")
            nc.tensor.matmul(ssp[:], ones31[:], spart[:], start=True,
                             stop=True)
            rcp = wp.tile([P, 1], F32, name=f"rcp{s}", tag="p31", bufs=4)
            nc.vector.reciprocal(rcp[:], ssp[:])
            xo = wp.tile([P, P], F32, name=f"xo{s}", tag=f"rst{s}", bufs=2)
            nc.vector.tensor_scalar(xo[:], x3[:], rcp[:], None, op0=MUL)
            nc.sync.dma_start(d_out[s], xo[:])

    nc.compile()
    return nc


def _get_program(n_iter=N_ITER):
    key = ("nc", n_iter)
    if key not in _PROGRAM_CACHE:
        _PROGRAM_CACHE[key] = _build_program(n_iter)
    return _PROGRAM_CACHE[key]


def _core_assignment(b, cch):
    pairs = [(bi, ci) for bi in range(b) for ci in range(cch)]
    ext = list(pairs)
    while len(ext) < NCORES * SLICES_PER_CORE:
        ext.append(pairs[len(ext) - len(pairs)])
    return [(ext[k], ext[k + NCORES]) for k in range(NCORES)]


def kernel(blurx, blury, latentx, latenty, psf_size):
    psf_size = int(np.asarray(psf_size))
    assert psf_size == P, f"kernel hardcoded for psf_size=31, got {psf_size}"
    blurx = np.asarray(blurx, dtype=np.float32)
    blury = np.asarray(blury, dtype=np.float32)
    latentx = np.asarray(latentx, dtype=np.float32)
    latenty = np.asarray(latenty, dtype=np.float32)
    b, cch, H, W = blurx.shape
    assert (H, W) == (N, N)
    nc = _get_program()
    consts = _make_consts()
    percore = _core_assignment(b, cch)
    in_maps = []
    for k in range(NCORES):
        m = dict(consts)
        for nm, arr in (("bx", blurx), ("by", blury),
                        ("lx", latentx), ("ly", latenty)):
            m[nm] = np.stack([_to_sb(arr[bi, ci]) for (bi, ci) in percore[k]])
        in_maps.append(m)
    from concourse.bass_utils import run_bass_kernel_spmd
    res = run_bass_kernel_spmd(nc, in_maps, core_ids=list(range(NCORES)))
    out = np.zeros((b, cch, P, P), np.float32)
    done = set()
    for k in range(NCORES):
        for j, (bi, ci) in enumerate(percore[k]):
            if (bi, ci) not in done:
                out[bi, ci] = res.results[k]["out"][j]
                done.add((bi, ci))
    return out


if __name__ == "__main__":
    d = np.load('/root/problem/ref_inputs.npz')
    out = kernel(d['blurx'], d['blury'], d['latentx'], d['latenty'], 31)
    ref = np.load('/root/problem/ref_out.npy')
    err = np.abs(out - ref)
    print("absmax rel:", err.max() / np.abs(ref).max())
    print("fro rel:", np.linalg.norm(out - ref) / np.linalg.norm(ref))


# revision 19
# speedup vs baseline: 2.9756x; 1.1065x over previous
"""Trainium2 Bass kernel for nn_EstimatePSF: FFT-based PSF estimation via CG.

Strategy:
- All 2D FFTs/IFFTs expressed as DFT matmuls on the TensorEngine (fp32).
  Rolls/pads/crops are absorbed into precomputed DFT-matrix constants.
- Data-parallel over the 12 (b,c) slices; SPMD over 8 cores, 2 slices per
  core (4 slices duplicated to fill 16 = 8*2 program slots). No collectives.
- All 512x512 spectra live TRANSPOSED ("spectrum layout"); the 31x31 CG
  state stays natural. crop-IFFT swaps lhsT/rhs in its last stage so the
  natural orientation comes back for free.
- r0 computed via linearity: D = bf - lft*xf0 (xf0 = analytic spectrum of
  the uniform init kernel, masked -> real), r0 = cropIFFT(D) - x0.
- The psf2otf imag-mask is computed with max|Im|, max|Re| reductions and
  applied by scaling the imag-term DFT constants by keep (0/1) - exact.

Self-contained: hardcodes shapes (4,3,512,512) f32, psf_size=31.
"""
import sys
import math as _math
import numpy as np

sys.path.insert(0, '/opt/trn_rl_repo')

P = 31
N = 512
EPS32 = 1.1920928955078125e-07
NOPS_T = np.float32(P * P * (2.0 * _math.log2(P)) * EPS32)
T2 = float(np.float32(np.float32(NOPS_T) * np.float32(NOPS_T)))
N_ITER = 10
NCORES = 8
SLICES_PER_CORE = 2


def _to_sb(a):
    """[512, X] row-major -> SBUF layout [128, 4X] (4 row-chunks side by side)."""
    X = a.shape[1]
    return np.ascontiguousarray(
        a.reshape(4, 128, X).transpose(1, 0, 2).reshape(128, 4 * X))


def _make_consts():
    k = np.arange(N)
    ang = -2.0 * np.pi * np.outer(k, k) / N
    Wr = np.cos(ang).astype(np.float32)   # symmetric
    Wi = np.sin(ang).astype(np.float32)
    i31 = np.arange(P) - (P // 2)
    angc = -2.0 * np.pi * np.outer(k, i31) / N   # [512, 31] : Wc
    WcTr = np.cos(angc).astype(np.float32).T.copy()  # [31, 512]
    WcTi = np.sin(angc).astype(np.float32).T.copy()
    angp = 2.0 * np.pi * np.outer(i31, k) / N    # [31, 512]
    Er = np.cos(angp).astype(np.float64)
    Ei = np.sin(angp).astype(np.float64)
    PlTr = (Er / (N * N)).astype(np.float32).T.copy()  # [512, 31]
    PlTi = (Ei / (N * N)).astype(np.float32).T.copy()
    PrTr = Er.astype(np.float32).T.copy()
    PrTi = Ei.astype(np.float32).T.copy()
    with np.errstate(invalid='ignore', divide='ignore'):
        D31 = np.sin(31 * np.pi * k / N) / np.sin(np.pi * k / N)
    D31[0] = 31.0
    xf0 = (np.outer(D31, D31) / (P * P)).astype(np.float32)
    # stacked 2-band constants for PE row-packing (band0 rows 0-30,
    # band1 rows 32-62; row 31/63 zero-padding)
    def stack2(a, b):
        out = np.zeros((63, a.shape[1]), np.float32)
        out[0:31] = a
        out[32:63] = b
        return out
    wcpa = stack2(WcTr, WcTi)            # xf pair-1 lhsT (rhs = ttr both bands)
    wcpb = stack2(-WcTi, WcTr)           # xf pair-2 lhsT (rhs = tti both bands)
    wcts = stack2(WcTr, WcTi)            # TT-step rhs stack (lhsT = p both bands)
    ident2 = stack2(np.eye(P, dtype=np.float32), np.eye(P, dtype=np.float32))
    return {
        "wr": _to_sb(Wr), "wi": _to_sb(Wi), "nwi": _to_sb(-Wi),
        "wcpa": wcpa, "wcpb": wcpb, "wcts": wcts, "ident2": ident2,
        "pltr": _to_sb(PlTr), "plti": _to_sb(PlTi), "nplti": _to_sb(-PlTi),
        "prtr": _to_sb(PrTr), "nprti": _to_sb(-PrTi),
        "xf0": _to_sb(xf0),
        "ident": np.eye(128, dtype=np.float32),
    }


_PROGRAM_CACHE = {}


def _build_program(n_iter=N_ITER, stage=99, sub=99):
    from contextlib import ExitStack
    import concourse.bacc as bacc
    import concourse.tile as tile
    from concourse import mybir
    from concourse.alu_op_type import AluOpType

    F32 = mybir.dt.float32
    AX = mybir.AxisListType
    MUL = AluOpType.mult
    ADD = AluOpType.add
    MAX = AluOpType.max

    nc = bacc.Bacc(None, target_bir_lowering=False, debug=False)

    # ---- DRAM ----
    d_in = {}
    for nm in ("bx", "by", "lx", "ly"):
        d_in[nm] = nc.dram_tensor(nm, [SLICES_PER_CORE, 128, 4 * N], F32,
                                  kind="ExternalInput").ap()
    d_c = {}
    for nm, shp in (("wr", [128, 4 * N]), ("wi", [128, 4 * N]),
                    ("nwi", [128, 4 * N]),
                    ("wcpa", [63, N]), ("wcpb", [63, N]), ("wcts", [63, N]),
                    ("ident2", [63, P]),
                    ("pltr", [128, 4 * P]), ("plti", [128, 4 * P]),
                    ("nplti", [128, 4 * P]),
                    ("prtr", [128, 4 * P]), ("nprti", [128, 4 * P]),
                    ("xf0", [128, 4 * N]), ("ident", [128, 128])):
        d_c[nm] = nc.dram_tensor(nm, shp, F32, kind="ExternalInput").ap()
    d_out = nc.dram_tensor("out", [SLICES_PER_CORE, P, P], F32,
                           kind="ExternalOutput").ap()

    with tile.TileContext(nc) as tc, ExitStack() as ctx:
        cp = ctx.enter_context(tc.tile_pool(name="consts", bufs=1))
        wp = ctx.enter_context(tc.tile_pool(name="work", bufs=1))
        pmm = ctx.enter_context(tc.tile_pool(name="pmm", bufs=4, space="PSUM"))
        ptc = ctx.enter_context(tc.tile_pool(name="ptc", bufs=2, space="PSUM"))
        psml = ctx.enter_context(tc.tile_pool(name="psml", bufs=2, space="PSUM"))

        # ---- constants to SBUF ----
        c = {}
        for nm in d_c:
            if nm == "xf0":
                continue  # streamed chunk-wise from DRAM
            c[nm] = cp.tile(list(d_c[nm].shape), F32, name=f"c_{nm}")
            nc.sync.dma_start(c[nm][:], d_c[nm][:])
        ones31 = cp.tile([P, P], F32, name="ones31")
        nc.vector.memset(ones31[:], 1.0)
        ones1x128 = cp.tile([1, 128], F32, name="ones1x128")
        nc.vector.memset(ones1x128[:], 1.0)

        BIG = [128, 4 * N]

        def big(name, tag, bufs=1):
            return wp.tile(BIG, F32, name=name, tag=tag, bufs=bufs)

        def chunk_t(name):
            return wp.tile([128, N], F32, name=name, tag="pch", bufs=4)

        # ---------- emit helpers ----------
        def fft2T_stage1(s, img, tag):
            """stage 1: UT = A^T @ W (psum->sbuf). Returns utr, uti [128,2048]."""
            utr = big(f"utr_{tag}{s}", "ut_r")
            uti = big(f"uti_{tag}{s}", "ut_i")
            for m in range(4):
                pr = pmm.tile([128, N], F32, name=f"p_ut_r{tag}{s}{m}", tag="pmm")
                pi = pmm.tile([128, N], F32, name=f"p_ut_i{tag}{s}{m}", tag="pmm")
                for rc in range(4):
                    lhs = img[:, rc * N + m * 128: rc * N + (m + 1) * 128]
                    nc.tensor.matmul(pr[:], lhs, c["wr"][:, rc * N:(rc + 1) * N],
                                     start=(rc == 0), stop=(rc == 3))
                for rc in range(4):
                    lhs = img[:, rc * N + m * 128: rc * N + (m + 1) * 128]
                    nc.tensor.matmul(pi[:], lhs, c["wi"][:, rc * N:(rc + 1) * N],
                                     start=(rc == 0), stop=(rc == 3))
                nc.scalar.copy(utr[:, m * N:(m + 1) * N], pr[:])
                nc.scalar.copy(uti[:, m * N:(m + 1) * N], pi[:])
            return utr, uti

        def stage2_chunk(prefix, s, mo, utr, uti):
            """stage 2 chunk mo: F^T[mo] in psum (pr, pi)."""
            pr = pmm.tile([128, N], F32, name=f"{prefix}r{s}{mo}", tag="pmm")
            pi = pmm.tile([128, N], F32, name=f"{prefix}i{s}{mo}", tag="pmm")
            for cc in range(4):
                lw = slice(cc * N + mo * 128, cc * N + (mo + 1) * 128)
                nc.tensor.matmul(pr[:], c["wr"][:, lw],
                                 utr[:, cc * N:(cc + 1) * N],
                                 start=(cc == 0), stop=False)
                nc.tensor.matmul(pr[:], c["nwi"][:, lw],
                                 uti[:, cc * N:(cc + 1) * N],
                                 start=False, stop=(cc == 3))
                nc.tensor.matmul(pi[:], c["wr"][:, lw],
                                 uti[:, cc * N:(cc + 1) * N],
                                 start=(cc == 0), stop=False)
                nc.tensor.matmul(pi[:], c["wi"][:, lw],
                                 utr[:, cc * N:(cc + 1) * N],
                                 start=False, stop=(cc == 3))
            return pr, pi

        def crop_ifft(s, gr, gi, lhs_ni, lhs_r2, tag):
            """yp psum [31,31] natural = Re(crop(ifft2(G))) from transposed
            spectrum G (gr, gi [128,2048] sbuf).
            lhs_ni: const/tile for -PlTi (Cr Gi-term); lhs_r2: PlTr for the
            Ci Gi-term (keep-scaled in CG)."""
            # 4-band col-packed C-step: Cr halves in array col-groups 0/64
            # (psum partitions 0-30 / 64-94), Ci halves in groups 32/96.
            # Each band accumulates 2 k1-chunks; 4 bands run concurrently.
            cpk = ptc.tile([127, N], F32, name=f"cpk{tag}{s}", tag="ptc")
            for cc in range(4):
                ls = slice(cc * P, (cc + 1) * P)
                rs = slice(cc * N, (cc + 1) * N)
                h = 0 if cc < 2 else 64          # Cr band offset
                first = (cc % 2 == 0)
                last = (cc % 2 == 1)
                nc.tensor.matmul(cpk[h:h + P, :], c["pltr"][:, ls], gr[:, rs],
                                 start=first, stop=False,
                                 tile_position=(0, h), skip_group_check=True)
                nc.tensor.matmul(cpk[h + 32:h + 32 + P, :], c["plti"][:, ls],
                                 gr[:, rs], start=first, stop=False,
                                 tile_position=(0, h + 32),
                                 skip_group_check=True)
                nc.tensor.matmul(cpk[h:h + P, :], lhs_ni[:, ls], gi[:, rs],
                                 start=False, stop=last,
                                 tile_position=(0, h), skip_group_check=True)
                nc.tensor.matmul(cpk[h + 32:h + 32 + P, :], lhs_r2[:, ls],
                                 gi[:, rs], start=False, stop=last,
                                 tile_position=(0, h + 32),
                                 skip_group_check=True)
            # combine halves: Cr = band0 + band2, Ci = band1 + band3
            tr2 = wp.tile([P, N], F32, name=f"tr2{tag}{s}", tag="csb", bufs=4)
            ti2 = wp.tile([P, N], F32, name=f"ti2{tag}{s}", tag="csb", bufs=4)
            nc.scalar.copy(tr2[:], cpk[64:64 + P, :])
            nc.scalar.copy(ti2[:], cpk[96:96 + P, :])
            cr_sb = wp.tile([P, N], F32, name=f"crsb{tag}{s}", tag="csb",
                            bufs=4)
            ci_sb = wp.tile([P, N], F32, name=f"cisb{tag}{s}", tag="csb",
                            bufs=4)
            nc.vector.tensor_add(cr_sb[:], cpk[0:31, :], tr2[:])
            nc.vector.tensor_add(ci_sb[:], cpk[32:63, :], ti2[:])
            if sub <= 61:
                dbg = wp.tile([P, P], F32, name=f"dbgs61{tag}{s}", tag="junk31", bufs=2)
                nc.vector.tensor_copy(dbg[:], cr_sb[:, :P])
                nc.sync.dma_start(d_out[s], dbg[:])
                return None
            ctp = psml.tile([128, 8 * P], F32, name=f"ctp{tag}{s}", tag="psml")
            for cc in range(4):
                nc.tensor.transpose(ctp[:, cc * P:(cc + 1) * P],
                                    cr_sb[:, cc * 128:(cc + 1) * 128],
                                    c["ident"][:P, :P])
                nc.tensor.transpose(ctp[:, (4 + cc) * P:(5 + cc) * P],
                                    ci_sb[:, cc * 128:(cc + 1) * 128],
                                    c["ident"][:P, :P])
            ct_sb = wp.tile([128, 8 * P], F32, name=f"ctsb{tag}{s}", tag="ctsb",
                            bufs=2)
            nc.scalar.copy(ct_sb[:], ctp[:])
            if sub <= 62:
                dbg = wp.tile([P, P], F32, name=f"dbgs62{tag}{s}", tag="junk31", bufs=2)
                nc.vector.tensor_copy(dbg[:], ct_sb[:P, :P])
                nc.sync.dma_start(d_out[s], dbg[:])
                return None
            yp = psml.tile([P, P], F32, name=f"yp{tag}{s}", tag="psml")
            for cc in range(4):
                nc.tensor.matmul(yp[:], c["prtr"][:, cc * P:(cc + 1) * P],
                                 ct_sb[:, cc * P:(cc + 1) * P],
                                 start=(cc == 0), stop=False)
                nc.tensor.matmul(yp[:], c["nprti"][:, cc * P:(cc + 1) * P],
                                 ct_sb[:, (4 + cc) * P:(5 + cc) * P],
                                 start=False, stop=(cc == 3))
            return yp

        def part_sum_bcast(s, a31, b31, tag):
            """sum(a*b) over [31,31] -> psum [31,1] broadcast on 31 partitions."""
            junk = wp.tile([P, P], F32, name=f"junk{tag}{s}", tag="junk31",
                           bufs=2)
            part = wp.tile([P, 1], F32, name=f"part{tag}{s}", tag="p31", bufs=4)
            nc.vector.tensor_mul(junk[:], a31[:], b31[:])
            nc.vector.tensor_reduce(part[:], junk[:], axis=AX.X, op=ADD)
            sp = psml.tile([P, 1], F32, name=f"sump{tag}{s}", tag="psml")
            nc.tensor.matmul(sp[:], ones31[:], part[:], start=True, stop=True)
            return sp

        # ---------- per-slice state ----------
        lft = [None] * SLICES_PER_CORE
        xs = [None] * SLICES_PER_CORE
        rs_ = [None] * SLICES_PER_CORE
        ps_ = [None] * SLICES_PER_CORE
        rsold = [None] * SLICES_PER_CORE

        # ---------- init phase (per slice; latent first, blur fused) ----------
        for s in range(SLICES_PER_CORE):
            # latent magnitude
            ax_ = big(f"rawlx{s}", "rawA")
            ay_ = big(f"rawly{s}", "rawB")
            nc.sync.dma_start(ax_[:], d_in["lx"][s])
            nc.sync.dma_start(ay_[:], d_in["ly"][s])
            u = big(f"lsqx{s}", "sq1")
            v = big(f"lsqy{s}", "sq2")
            nc.scalar.square(u[:], ax_[:])
            nc.scalar.square(v[:], ay_[:])
            lat = big(f"lat{s}", "img")
            nc.vector.tensor_add(lat[:], u[:], v[:])
            nc.scalar.sqrt(lat[:], lat[:])
            if stage <= 1:
                dbg = wp.tile([P, P], F32, name=f"dbg1_{s}", tag="junk31", bufs=2)
                nc.vector.tensor_copy(dbg[:], lat[:P, :P])
                nc.sync.dma_start(d_out[s], dbg[:])
                continue
            # latent FFT -> fltr, flti in SBUF
            utr, uti = fft2T_stage1(s, lat, "l")
            if stage <= 2:
                dbg = wp.tile([P, P], F32, name=f"dbg2_{s}", tag="junk31", bufs=2)
                nc.vector.tensor_copy(dbg[:], utr[:P, :P])
                nc.sync.dma_start(d_out[s], dbg[:])
                continue
            fltr = big(f"fltr{s}", "fl_r")
            flti = big(f"flti{s}", "fl_i")
            for mo in range(4):
                pr, pi = stage2_chunk("p_fl", s, mo, utr, uti)
                nc.scalar.copy(fltr[:, mo * N:(mo + 1) * N], pr[:])
                nc.scalar.copy(flti[:, mo * N:(mo + 1) * N], pi[:])
            if stage <= 3:
                dbg = wp.tile([P, P], F32, name=f"dbg3_{s}", tag="junk31", bufs=2)
                nc.vector.tensor_copy(dbg[:], fltr[:P, :P])
                nc.sync.dma_start(d_out[s], dbg[:])
                continue
            # lft = fltr^2 + flti^2
            u2 = big(f"lftsq1{s}", "sq1")
            v2 = big(f"lftsq2{s}", "sq2")
            nc.scalar.square(u2[:], fltr[:])
            nc.scalar.square(v2[:], flti[:])
            lft[s] = wp.tile(BIG, F32, name=f"lft{s}", tag=f"lft{s}", bufs=1)
            nc.vector.tensor_add(lft[s][:], u2[:], v2[:])
            if stage <= 4:
                dbg = wp.tile([P, P], F32, name=f"dbg4_{s}", tag="junk31", bufs=2)
                nc.vector.tensor_copy(dbg[:], lft[s][:P, :P])
                nc.sync.dma_start(d_out[s], dbg[:])
                continue
            # blur magnitude
            bx_ = big(f"rawbx{s}", "rawA")
            by_ = big(f"rawby{s}", "rawB")
            nc.sync.dma_start(bx_[:], d_in["bx"][s])
            nc.sync.dma_start(by_[:], d_in["by"][s])
            ub = big(f"bsqx{s}", "sq1")
            vb = big(f"bsqy{s}", "sq2")
            nc.scalar.square(ub[:], bx_[:])
            nc.scalar.square(vb[:], by_[:])
            blur = big(f"blur{s}", "img")
            nc.vector.tensor_add(blur[:], ub[:], vb[:])
            nc.scalar.sqrt(blur[:], blur[:])
            # blur FFT with fused D products (blur spectrum never hits SBUF)
            butr, buti = fft2T_stage1(s, blur, "b")
            dr = big(f"dr_{s}", "dd_r")
            di = big(f"di_{s}", "dd_i")
            for mo in range(4):
                pr, pi = stage2_chunk("p_fb", s, mo, butr, buti)
                rng = slice(mo * N, (mo + 1) * N)
                m1 = chunk_t(f"m1_{s}{mo}")
                m2 = chunk_t(f"m2_{s}{mo}")
                nc.vector.tensor_mul(m1[:], fltr[:, rng], pr[:])
                nc.vector.tensor_mul(m2[:], flti[:, rng], pi[:])
                nc.vector.tensor_add(dr[:, rng], m1[:], m2[:])
                nc.vector.tensor_mul(m1[:], fltr[:, rng], pi[:])
                nc.vector.tensor_mul(m2[:], flti[:, rng], pr[:])
                nc.vector.tensor_sub(di[:, rng], m1[:], m2[:])
                # Dr -= lft * xf0   (xf0 chunk streamed from DRAM)
                xq = chunk_t(f"xq_{s}{mo}")
                nc.sync.dma_start(xq[:], d_c["xf0"][:, rng])
                nc.vector.tensor_mul(xq[:], lft[s][:, rng], xq[:])
                nc.vector.tensor_sub(dr[:, rng], dr[:, rng], xq[:])
            if stage <= 6:
                dbg = wp.tile([P, P], F32, name=f"dbg6_{s}", tag="junk31", bufs=2)
                nc.vector.tensor_copy(dbg[:], dr[:P, :P])
                nc.sync.dma_start(d_out[s], dbg[:])
                continue
            # r0 = cropIFFT(D) - 1/961 ; p0 = r0 ; x0 = 1/961 ; rsold
            yp = crop_ifft(s, dr, di, c["nplti"], c["pltr"], tag="r0")
            if yp is None:
                continue
            if sub <= 63:
                dbg = wp.tile([P, P], F32, name=f"dbgs63{s}", tag="junk31", bufs=2)
                nc.vector.tensor_copy(dbg[:], yp[:])
                nc.sync.dma_start(d_out[s], dbg[:])
                continue
            r0 = wp.tile([P, P], F32, name=f"r_{s}", tag=f"rst{s}", bufs=2)
            nc.vector.tensor_scalar(r0[:], yp[:], -1.0 / (P * P), None, op0=ADD)
            rs_[s] = r0
            p0 = wp.tile([P, P], F32, name=f"p_{s}", tag=f"pst{s}", bufs=2)
            nc.vector.tensor_copy(p0[:], r0[:])
            ps_[s] = p0
            if sub <= 64:
                nc.sync.dma_start(d_out[s], r0[:])
                continue
            x0 = wp.tile([P, P], F32, name=f"x_{s}", tag=f"xst{s}", bufs=2)
            nc.vector.memset(x0[:], 1.0 / (P * P))
            xs[s] = x0
            sp = part_sum_bcast(s, r0, r0, "rs0")
            rso = wp.tile([P, 1], F32, name=f"rsold{s}", tag=f"rso{s}", bufs=2)
            nc.vector.tensor_copy(rso[:], sp[:])
            rsold[s] = rso

        # ---------- CG iterations ----------
        if stage == 7 and rs_[0] is not None:
            for s in range(SLICES_PER_CORE):
                nc.sync.dma_start(d_out[s], rs_[s][:])
        for it in range(n_iter if stage > 7 else 0):
            last = (it == n_iter - 1)
            for s in range(SLICES_PER_CORE):
                p_s = ps_[s]
                # step A (row-packed pair): TTr = p^T@WcTr (band0),
                # TTi = p^T@WcTi (band1). lhsT = p stacked at both bands.
                pstk = wp.tile([63, P], F32, name=f"pstk{s}_{it}", tag="pstk",
                               bufs=4)
                nc.scalar.copy(pstk[0:31, :], p_s[:])
                nc.scalar.copy(pstk[32:63, :], p_s[:])
                ttrp = ptc.tile([P, N], F32, name=f"ttrp{s}_{it}", tag="ptc")
                ttip = ptc.tile([P, N], F32, name=f"ttip{s}_{it}", tag="ptc")
                nc.tensor.matmul(ttrp[:], pstk[0:31, :], c["wcts"][0:31, :],
                                 start=True, stop=True, tile_position=(0, 0))
                nc.tensor.matmul(ttip[:], pstk[32:63, :], c["wcts"][32:63, :],
                                 start=True, stop=True, tile_position=(32, 0))
                # TT to SBUF, stacked twice for the row-packed xf step:
                # tt_rr = [ttr; ttr], tt_ii = [tti; tti]
                tt_rr = wp.tile([63, N], F32, name=f"ttrr{s}_{it}", tag="ttsb",
                                bufs=4)
                tt_ii = wp.tile([63, N], F32, name=f"ttii{s}_{it}", tag="ttsb",
                                bufs=4)
                nc.scalar.copy(tt_rr[0:31, :], ttrp[:])
                nc.scalar.copy(tt_rr[32:63, :], ttrp[:])
                nc.scalar.copy(tt_ii[0:31, :], ttip[:])
                nc.scalar.copy(tt_ii[32:63, :], ttip[:])
                # step B: xf' chunks + mask partials + products
                gr = big(f"gr{s}_{it}", "g_r", bufs=2)
                gi = big(f"gi{s}_{it}", "g_i", bufs=2)
                mip = wp.tile([128, 4], F32, name=f"mip{s}_{it}", tag="mp128",
                              bufs=4)
                mrp = wp.tile([128, 4], F32, name=f"mrp{s}_{it}", tag="mp128",
                              bufs=4)
                for cc in range(4):
                    xrp = pmm.tile([128, N], F32, name=f"xrp{s}_{it}{cc}",
                                   tag="pmm")
                    xip = pmm.tile([128, N], F32, name=f"xip{s}_{it}{cc}",
                                   tag="pmm")
                    lw = slice(cc * 128, (cc + 1) * 128)
                    # row-packed pairs: (xr+=WcTr@ttr | xi+=WcTi@ttr) then
                    # (xr+=-WcTi@tti | xi+=WcTr@tti); banks differ per pair.
                    nc.tensor.matmul(xrp[:], c["wcpa"][0:31, lw],
                                     tt_rr[0:31, :], start=True, stop=False,
                                     tile_position=(0, 0))
                    nc.tensor.matmul(xip[:], c["wcpa"][32:63, lw],
                                     tt_rr[32:63, :], start=True, stop=False,
                                     tile_position=(32, 0))
                    nc.tensor.matmul(xrp[:], c["wcpb"][0:31, lw],
                                     tt_ii[0:31, :], start=False, stop=True,
                                     tile_position=(0, 0))
                    nc.tensor.matmul(xip[:], c["wcpb"][32:63, lw],
                                     tt_ii[32:63, :], start=False, stop=True,
                                     tile_position=(32, 0))
                    nc.vector.tensor_reduce(mip[:, cc:cc + 1], xip[:],
                                            axis=AX.X, op=MAX,
                                            apply_absolute_value=True)
                    nc.vector.tensor_reduce(mrp[:, cc:cc + 1], xrp[:],
                                            axis=AX.X, op=MAX,
                                            apply_absolute_value=True)
                    rng = slice(cc * N, (cc + 1) * N)
                    nc.vector.tensor_mul(gr[:, rng], lft[s][:, rng], xrp[:])
                    nc.vector.tensor_mul(gi[:, rng], lft[s][:, rng], xip[:])
                # mask: keep = (mi^2 > t^2*(mi^2+mr^2))
                mi1 = wp.tile([128, 1], F32, name=f"mi1{s}_{it}", tag="k128",
                              bufs=4)
                mr1 = wp.tile([128, 1], F32, name=f"mr1{s}_{it}", tag="k128",
                              bufs=4)
                nc.vector.tensor_reduce(mi1[:], mip[:], axis=AX.X, op=MAX)
                nc.vector.tensor_reduce(mr1[:], mrp[:], axis=AX.X, op=MAX)
                trm = psml.tile([1, 256], F32, name=f"trm{s}_{it}", tag="psml")
                nc.tensor.transpose(trm[:, 0:128], mi1[:], c["ident"][:])
                nc.tensor.transpose(trm[:, 128:256], mr1[:], c["ident"][:])
                mis = wp.tile([1, 4], F32, name=f"mis{s}_{it}", tag="s14",
                              bufs=4)
                nc.vector.tensor_reduce(mis[:, 0:1], trm[:, 0:128], axis=AX.X,
                                        op=MAX)
                nc.vector.tensor_reduce(mis[:, 1:2], trm[:, 128:256],
                                        axis=AX.X, op=MAX)
                nc.vector.tensor_mul(mis[:, 2:3], mis[:, 0:1], mis[:, 0:1])
                nc.vector.tensor_mul(mis[:, 3:4], mis[:, 1:2], mis[:, 1:2])
                keep = wp.tile([1, 2], F32, name=f"keep{s}_{it}", tag="s14",
                               bufs=4)
                nc.vector.tensor_add(keep[:, 1:2], mis[:, 2:3], mis[:, 3:4])
                nc.vector.tensor_scalar(keep[:, 1:2], keep[:, 1:2], T2, None,
                                        op0=MUL)
                nc.vector.tensor_tensor(keep[:, 0:1], mis[:, 2:3],
                                        keep[:, 1:2], op=AluOpType.is_gt)
                kbp = psml.tile([128, 1], F32, name=f"kbp{s}_{it}", tag="psml")
                nc.tensor.matmul(kbp[:], ones1x128[:], keep[:, 0:1],
                                 start=True, stop=True)
                ksb = wp.tile([128, 1], F32, name=f"ksb{s}_{it}", tag="k128",
                              bufs=4)
                nc.vector.tensor_copy(ksb[:], kbp[:])
                pltr_k = wp.tile([128, 4 * P], F32, name=f"pltrk{s}_{it}",
                                 tag="plk", bufs=2)
                nplti_k = wp.tile([128, 4 * P], F32, name=f"npltik{s}_{it}",
                                  tag="nplk", bufs=2)
                nc.vector.tensor_scalar(pltr_k[:], c["pltr"][:], ksb[:], None,
                                        op0=MUL)
                nc.vector.tensor_scalar(nplti_k[:], c["nplti"][:], ksb[:],
                                        None, op0=MUL)
                # steps C/D: Ap = Re(crop(ifft(G))) + p
                yp = crop_ifft(s, gr, gi, nplti_k, pltr_k, tag=f"cg{it}")
                ap_sb = wp.tile([P, P], F32, name=f"ap{s}_{it}", tag="apsb",
                                bufs=2)
                nc.vector.tensor_add(ap_sb[:], yp[:], p_s[:])
                # CG update
                dnp = part_sum_bcast(s, p_s, ap_sb, f"dn{it}")
                alpha = wp.tile([P, 2], F32, name=f"alph{s}_{it}", tag="p31x2",
                                bufs=4)
                nc.vector.reciprocal(alpha[:, 1:2], dnp[:])
                nc.vector.tensor_mul(alpha[:, 0:1], rsold[s][:], alpha[:, 1:2])
                nc.vector.tensor_scalar(alpha[:, 1:2], alpha[:, 0:1], -1.0,
                                        None, op0=MUL)
                xn = wp.tile([P, P], F32, name=f"x_{s}_{it}", tag=f"xst{s}",
                             bufs=2)
                nc.vector.scalar_tensor_tensor(xn[:], p_s[:], alpha[:, 0:1],
                                               xs[s][:], op0=MUL, op1=ADD)
                xs[s] = xn
                if not last:
                    rn = wp.tile([P, P], F32, name=f"r_{s}_{it}",
                                 tag=f"rst{s}", bufs=2)
                    nc.vector.scalar_tensor_tensor(rn[:], ap_sb[:],
                                                   alpha[:, 1:2], rs_[s][:],
                                                   op0=MUL, op1=ADD)
                    rs_[s] = rn
                    rsp = part_sum_bcast(s, rn, rn, f"rs{it}")
                    rsn = wp.tile([P, 1], F32, name=f"rsold{s}_{it}",
                                  tag=f"rso{s}", bufs=2)
                    nc.vector.tensor_copy(rsn[:], rsp[:])
                    beta = wp.tile([P, 2], F32, name=f"beta{s}_{it}",
                                   tag="p31x2", bufs=4)
                    nc.vector.reciprocal(beta[:, 1:2], rsold[s][:])
                    nc.vector.tensor_mul(beta[:, 0:1], rsn[:], beta[:, 1:2])
                    pn = wp.tile([P, P], F32, name=f"p_{s}_{it}",
                                 tag=f"pst{s}", bufs=2)
                    nc.vector.scalar_tensor_tensor(pn[:], p_s[:],
                                                   beta[:, 0:1], rn[:],
                                                   op0=MUL, op1=ADD)
                    ps_[s] = pn
                    rsold[s] = rsn

        # ---------- finalize ----------
        for s in range(SLICES_PER_CORE if stage > 7 else 0):
            x = xs[s]
            xmp = wp.tile([P, 1], F32, name=f"xmp{s}", tag="p31", bufs=4)
            nc.vector.tensor_reduce(xmp[:], x[:], axis=AX.X, op=MAX)
            trx = psml.tile([1, P], F32, name=f"trx{s}", tag="psml")
            nc.tensor.transpose(trx[:], xmp[:], c["ident"][:P, :P])
            mx = wp.tile([1, 1], F32, name=f"mx{s}", tag="s14", bufs=4)
            nc.vector.tensor_reduce(mx[:], trx[:], axis=AX.X, op=MAX)
            nc.vector.tensor_scalar(mx[:], mx[:], 0.05, None, op0=MUL)
            thp = psml.tile([P, 1], F32, name=f"thp{s}", tag="psml")
            nc.tensor.matmul(thp[:], ones31[0:1, :], mx[:], start=True,
                             stop=True)
            thr = wp.tile([P, 1], F32, name=f"thr{s}", tag="p31", bufs=4)
            nc.vector.tensor_copy(thr[:], thp[:])
            km = wp.tile([P, P], F32, name=f"km{s}", tag="junk31", bufs=2)
            nc.vector.tensor_scalar(km[:], x[:], thr[:], None,
                                    op0=AluOpType.is_ge)
            x2 = wp.tile([P, P], F32, name=f"x2_{s}", tag=f"xst{s}", bufs=2)
            nc.vector.tensor_mul(x2[:], x[:], km[:])
            x3 = wp.tile([P, P], F32, name=f"x3_{s}", tag=f"pst{s}", bufs=2)
            nc.vector.tensor_scalar(x3[:], x2[:], 0.0, None, op0=MAX)
            spart = wp.tile([P, 1], F32, name=f"spart{s}", tag="p31", bufs=4)
            nc.vector.tensor_reduce(spart[:], x3[:], axis=AX.X, op=ADD)
            ssp = psml.tile([P, 1], F32, name=f"ssp{s}", tag="psml")
            nc.tensor.matmul(ssp[:], ones31[:], spart[:], start=True,
                             stop=True)
            rcp = wp.tile([P, 1], F32, name=f"rcp{s}", tag="p31", bufs=4)
            nc.vector.reciprocal(rcp[:], ssp[:])
            xo = wp.tile([P, P], F32, name=f"xo{s}", tag=f"rst{s}", bufs=2)
            nc.vector.tensor_scalar(xo[:], x3[:], rcp[:], None, op0=MUL)
            nc.sync.dma_start(d_out[s], xo[:])

    nc.compile()
    return nc


def _get_program(n_iter=N_ITER):
    key = ("nc", n_iter)
    if key not in _PROGRAM_CACHE:
        _PROGRAM_CACHE[key] = _build_program(n_iter)
    return _PROGRAM_CACHE[key]


def _core_assignment(b, cch):
    pairs = [(bi, ci) for bi in range(b) for ci in range(cch)]
    ext = list(pairs)
    while len(ext) < NCORES * SLICES_PER_CORE:
        ext.append(pairs[len(ext) - len(pairs)])
    return [(ext[k], ext[k + NCORES]) for k in range(NCORES)]


def kernel(blurx, blury, latentx, latenty, psf_size):
    psf_size = int(np.asarray(psf_size))
    assert psf_size == P, f"kernel hardcoded for psf_size=31, got {psf_size}"
    blurx = np.asarray(blurx, dtype=np.float32)
    blury = np.asarray(blury, dtype=np.float32)
    latentx = np.asarray(latentx, dtype=np.float32)
    latenty = np.asarray(latenty, dtype=np.float32)
    b, cch, H, W = blurx.shape
    assert (H, W) == (N, N)
    nc = _get_program()
    consts = _make_consts()
    percore = _core_assignment(b, cch)
    in_maps = []
    for k in range(NCORES):
        m = dict(consts)
        for nm, arr in (("bx", blurx), ("by", blury),
                        ("lx", latentx), ("ly", latenty)):
            m[nm] = np.stack([_to_sb(arr[bi, ci]) for (bi, ci) in percore[k]])
        in_maps.append(m)
    from concourse.bass_utils import run_bass_kernel_spmd
    res = run_bass_kernel_spmd(nc, in_maps, core_ids=list(range(NCORES)))
    out = np.zeros((b, cch, P, P), np.float32)
    done = set()
    for k in range(NCORES):
        for j, (bi, ci) in enumerate(percore[k]):
            if (bi, ci) not in done:
                out[bi, ci] = res.results[k]["out"][j]
                done.add((bi, ci))
    return out


if __name__ == "__main__":
    d = np.load('/root/problem/ref_inputs.npz')
    out = kernel(d['blurx'], d['blury'], d['latentx'], d['latenty'], 31)
    ref = np.load('/root/problem/ref_out.npy')
    err = np.abs(out - ref)
    print("absmax rel:", err.max() / np.abs(ref).max())
    print("fro rel:", np.linalg.norm(out - ref) / np.linalg.norm(ref))
